# revision 46
# baseline (speedup 1.0000x reference)
"""GATModelVAE (2-layer GAT encoder VAE, eval mode) on 8 Trainium2 NeuronCores.

Strategy: destination-node (graph) parallelism. Nodes are packed into
160 windows of 128 dst nodes (degree-sorted, banded so all 8 cores run an
identical program). Per window, incoming edges live in an ELL (slot-major)
layout: slot j of partition n is the j-th in-edge of window-node n; padded
slots point at a sentinel table row whose att-logit columns are -1e4 so
exp() gives exactly 0. Per-edge source features arrive via dma_gather from
an AllGather-replicated table (payload stored c-major i.e. head-minor, and
fp8 for layer 1, converted to f16 on the scalar engine so the DVE alpha-
weighting multiply runs in its fast packed-16-bit 2x mode). The weighting
is one in-place DVE multiply per chunk; the segment sum over edge slots is
one DVE pair-add level followed by PSUM-accumulated identity matmuls (half
the matmul count of slot-at-a-time accumulation), with the exp columns
riding along to yield the softmax denominators. The table build runs in
bf16, and both tables' AllGathers are split into band-blocks scheduled to
hide behind the CC barrier (table 1) and the pass-A window tail (table 2).
Softmax normalization (constant per destination node) is applied after
aggregation in the window epilogue.
"""

import sys

sys.path.insert(0, "/opt/trn_rl_repo")

import numpy as np
import ml_dtypes

N = 20000
E0 = 320000
FIN = 256
H1 = 64
H2 = 32
HEADS = 5
NEG = 0.2

NCORE = 8
P = 128
NWIN = 160            # global windows
NB = NWIN // NCORE    # windows (bands) per core: 20
MLOC = NB * P         # node slots per core: 2560
CONTRIB = MLOC        # per-core AG contribution rows
TROWS = NCORE * MLOC + 8   # + locally-written sentinel row (pad to 8)
SENT = NCORE * MLOC   # sentinel table row
WB = 384              # matmul row width (f32 elems) for the table-build PSUM
WBB = 512             # gathered table-1 row width in BYTES (fp8 payload; %256)
SLOT_CAP = 8          # max ELL slots per gather chunk (1024 idx = 64-desc packet cap)
# exp() is stored in f16 and pair-summed; a constant bias of -ln(16) on the
# exponent scales all numerators AND denominators by 1/16 (cancels in the
# softmax) giving 16x overflow headroom in the f16 partial sums.
EXP_BIAS = -2.772588722239781
# AllGather band-blocks. The first collective can't start before the global
# CC barrier (~50us), so AG1 uses two big blocks; AG2 is front-loaded with a
# small tail so pass B isn't stuck behind a large final AllGather.
BLOCKS1 = ((0, 10), (10, 20))
BLOCKS2 = ((0, 9), (9, 15), (15, 20))

_compiled = None  # (key, nc)
TRACE = False          # set True (e.g. from test.py) to capture an NTFF profile
TRACE_DIR = None       # optional dir for trace artifacts
LAST_RESULTS = None    # BassKernelResults of the most recent run


# ----------------------------------------------------------------------------
# host-side graph preparation
# ----------------------------------------------------------------------------
def _prep_graph(edge_index):
    src = np.concatenate([edge_index[0], np.arange(N, dtype=np.int64)])
    dst = np.concatenate([edge_index[1], np.arange(N, dtype=np.int64)])
    EE = src.shape[0]
    deg = np.bincount(dst, minlength=N)

    order = np.argsort(-deg, kind="stable")      # nodes by degree desc
    pos = np.empty(N, np.int64)
    pos[order] = np.arange(N)
    win = pos // P                               # global window id
    slot = pos % P
    core = win % NCORE
    band = win // NCORE

    # slots per band = max degree in band (shared by all 8 cores)
    D_band = np.zeros(NB, np.int64)
    np.maximum.at(D_band, band, deg)
    D_band = np.maximum(D_band, 1)

    # table rows follow the blocked AG layouts of BLOCKS1 / BLOCKS2
    def blocked_rows(blocks):
        tr = np.empty(N, np.int64)
        for (s, e) in blocks:
            m = (band >= s) & (band < e)
            tr[m] = (NCORE * s * P + core[m] * (e - s) * P
                     + (band[m] - s) * P + slot[m])
        return tr

    trow1 = blocked_rows(BLOCKS1)
    trow2 = blocked_rows(BLOCKS2)

    # per-edge ELL coordinates: (core, band, slot of dst, j = rank among dst's edges)
    eorder = np.argsort(dst, kind="stable")
    ds = dst[eorder]
    run_start = np.r_[0, np.flatnonzero(ds[1:] != ds[:-1]) + 1]
    j_in = np.arange(EE) - np.repeat(run_start, np.diff(np.r_[run_start, EE]))
    es, ed = src[eorder], ds

    ec, eb, eslot = core[ed], band[ed], slot[ed]

    # global chunk layout: chunks of exactly SLOT_CAP slots, crossing band
    # boundaries; each chunk is a list of (band, j0, n_slots) segments
    chunks = []
    cur, cap = [], SLOT_CAP
    for k in range(NB):
        d, j = int(D_band[k]), 0
        while d > 0:
            t = min(cap, d)
            cur.append((k, j, t))
            j += t
            d -= t
            cap -= t
            if cap == 0:
                chunks.append(cur)
                cur, cap = [], SLOT_CAP
    if cur:
        chunks.append(cur)

    # build per-core wrapped int16 index tensors
    idx_cols = sum(8 * sum(s[2] for s in ch) for ch in chunks)

    def build_idx(trow):
        esrc_row = trow[es].astype(np.int32)
        idx_all = np.full((NCORE, 16, idx_cols), SENT, np.int16)
        ell = {}
        for k in range(NB):
            a = np.full((NCORE, int(D_band[k]), P), SENT, np.int32)
            m = eb == k
            a[ec[m], j_in[m], eslot[m]] = esrc_row[m]
            ell[k] = a
        col = 0
        for ch in chunks:
            blk = np.concatenate(
                [ell[k][:, j0 : j0 + dn, :] for (k, j0, dn) in ch], axis=1
            ).reshape(NCORE, -1)
            d_c = sum(s[2] for s in ch)
            wrapped = blk.reshape(NCORE, -1, 16).transpose(0, 2, 1)
            idx_all[:, :, col : col + 8 * d_c] = wrapped.astype(np.int16)
            col += 8 * d_c
        assert col == idx_cols
        return np.tile(idx_all, (1, 8, 1))

    meta = dict(
        chunks=chunks, idx_cols=idx_cols, core=core, band=band, slot=slot,
        D_band=tuple(int(x) for x in D_band),
    )
    return build_idx(trow1), build_idx(trow2), meta


def _w_aug(W, att_s, att_d, heads, hc):
    fin = W.shape[0]
    Wr = W.reshape(fin, heads, hc)
    ws = np.einsum("fhc,hc->fh", Wr, att_s)
    wd = np.einsum("fhc,hc->fh", Wr, att_d)
    return ws.astype(np.float32), wd.astype(np.float32)


def _cmajor(W, heads, hc):
    # [fin, heads*hc] -> columns reordered so col (c*heads + h) = W[:, h*hc + c]
    fin = W.shape[0]
    return np.ascontiguousarray(
        W.reshape(fin, heads, hc).transpose(0, 2, 1).reshape(fin, heads * hc))


# ----------------------------------------------------------------------------
# device program
# ----------------------------------------------------------------------------
def _build_program(chunks, idx_cols, D_band):
    import concourse.bass as bass
    import concourse.bacc as bacc
    import concourse.mybir as mybir
    import concourse.tile as tile
    from concourse import library_config
    from concourse.masks import make_identity

    f32 = mybir.dt.float32
    bf16 = mybir.dt.bfloat16
    f16 = mybir.dt.float16
    f8 = mybir.dt.float8e4
    i8 = mybir.dt.int8
    AF = mybir.ActivationFunctionType
    OP = mybir.AluOpType

    nc = bacc.Bacc("TRN2", target_bir_lowering=False, debug=False,
                   num_devices=NCORE, num_swdge_queues=4)

    xT_d = nc.dram_tensor("xT", [FIN, MLOC], bf16, kind="ExternalInput").ap()
    w1_d = nc.dram_tensor("w1big", [FIN, WB], bf16, kind="ExternalInput").ap()
    w2_d = nc.dram_tensor("w2big", [H1, WB], f32, kind="ExternalInput").ap()
    sent_d = nc.dram_tensor("sent", [1, WBB], i8, kind="ExternalInput").ap()
    sent2_d = nc.dram_tensor("sent2", [1, WB], f16, kind="ExternalInput").ap()
    idx1_d = nc.dram_tensor("idx1", [P, idx_cols], mybir.dt.int16,
                            kind="ExternalInput").ap()
    idx2_d = nc.dram_tensor("idx2", [P, idx_cols], mybir.dt.int16,
                            kind="ExternalInput").ap()
    b1_d = nc.dram_tensor("b1r", [P, H1], f32, kind="ExternalInput").ap()
    b23_d = nc.dram_tensor("b23r", [P, 2 * H2], f32, kind="ExternalInput").ap()

    out_d = nc.dram_tensor("out", [MLOC, 2 * H2], f32,
                           kind="ExternalOutput").ap()

    dum_i = nc.dram_tensor("dumi", [8, 32], i8).ap()
    dum_o = nc.dram_tensor("dumo", [64, 32], i8, addr_space="Shared").ap()
    con1_d = nc.dram_tensor("contrib1", [CONTRIB, WBB], i8).ap()
    con2_d = nc.dram_tensor("contrib2", [CONTRIB, WB], f16).ap()
    tbl1_d = nc.dram_tensor("tbl1", [TROWS, WBB], i8, addr_space="Shared").ap()
    tbl2_d = nc.dram_tensor("tbl2", [TROWS, WB], f16, addr_space="Shared").ap()

    rg = [list(range(NCORE))]

    S_TOT = sum(D_band)                  # total ELL slot columns (352-ish)
    slot0 = [0] * NB                     # first global slot column of band k
    for k in range(1, NB):
        slot0[k] = slot0[k - 1] + D_band[k - 1]

    WA = 328                             # wt col stride pass A (325 used)
    WB2 = 336                            # wt col stride pass B (330 used)
    TWA, TWB = 325, 330                  # tree widths

    with tile.TileContext(nc) as tc:
        with (
            tc.tile_pool(name="const", bufs=1) as cpool,
            tc.tile_pool(name="resid", bufs=1) as rpool,
            tc.tile_pool(name="io", bufs=3) as iopool,
            tc.tile_pool(name="psum", bufs=3, space="PSUM") as pspool,
            tc.tile_pool(name="psumT", bufs=1, space="PSUM") as ptpool,
            tc.tile_pool(name="psumA", bufs=4, space="PSUM") as papool,
        ):
            nc.gpsimd.load_library(library_config.mlp)
            # a tiny dummy AllGather absorbs the one-time CC barrier + DMA
            # ring ramp so the first real AllGather starts without delay
            nc.gpsimd.collective_compute(
                "AllGather", mybir.AluOpType.bypass, replica_groups=rg,
                ins=[dum_i[:]], outs=[dum_o[:]])

            ident = cpool.tile([P, P], f32)
            make_identity(nc, ident[:])
            ident_t = cpool.tile([P, P], f16)
            nc.vector.tensor_copy(ident_t[:], ident[:])
            ebias = cpool.tile([P, 1], f32)
            nc.gpsimd.memset(ebias[:], EXP_BIAS)

            w1_t = cpool.tile([P, 2, WB], bf16)
            nc.sync.dma_start(w1_t[:], w1_d[:].rearrange("(k p) n -> p k n", p=P))
            w2_t = cpool.tile([H1, WB], f32)
            nc.sync.dma_start(w2_t[:], w2_d[:])
            sent_t = cpool.tile([1, WBB], i8)
            nc.sync.dma_start(sent_t[:], sent_d[:])
            sent2_t = cpool.tile([1, WB], f16)
            nc.sync.dma_start(sent2_t[:], sent2_d[:])
            b1_t = cpool.tile([P, H1], f32)
            nc.sync.dma_start(b1_t[:], b1_d[:])
            b23_t = cpool.tile([P, 2 * H2], f32)
            nc.sync.dma_start(b23_t[:], b23_d[:])

            idx1_t = rpool.tile([P, idx_cols], mybir.dt.int16)
            nc.scalar.dma_start(idx1_t[:], idx1_d[:])
            idx2_t = rpool.tile([P, idx_cols], mybir.dt.int16)
            nc.scalar.dma_start(idx2_t[:], idx2_d[:])
            xtpool_cm = tc.tile_pool(name="xt", bufs=1)
            xtpool = xtpool_cm.__enter__()
            xt_all = xtpool.tile([P, 2, MLOC], bf16)
            nc.sync.dma_start(xt_all[:], xT_d[:].rearrange("(k p) n -> p k n", p=P))

            ad1 = rpool.tile([P, NB, 5], f32)
            ad23 = rpool.tile([P, NB, 10], f32)
            h1T = rpool.tile([H1, MLOC], f32)

            # ---------------- pass A: layer-1 table -------------------------
            nc.sync.dma_start(tbl1_d[SENT : SENT + 1, :], sent_t[:])
            nc.sync.dma_start(tbl2_d[SENT : SENT + 1, :], sent2_t[:])
            for m in range(NB):
                ps = pspool.tile([P, WB], f32, space="PSUM", tag="xwps")
                for kk in range(2):
                    nc.tensor.matmul(ps[:], xt_all[:, kk, m * P : (m + 1) * P],
                                     w1_t[:, kk, :],
                                     start=(kk == 0), stop=(kk == 1))
                row_t = iopool.tile([P, WBB], i8, tag="rowt")
                nc.scalar.activation(row_t[:, 0:320].bitcast(f8), ps[:, 0:320],
                                     AF.Copy)
                nc.vector.tensor_copy(row_t[:, 320:330].bitcast(f16),
                                      ps[:, 320:325])
                nc.vector.tensor_copy(ad1[:, m, :], ps[:, 325:330])
                nc.sync.dma_start(con1_d[m * P : (m + 1) * P, :], row_t[:])
                for (s, e) in BLOCKS1:
                    if m == e - 1:
                        nc.gpsimd.collective_compute(
                            "AllGather", mybir.AluOpType.bypass,
                            replica_groups=rg,
                            ins=[con1_d[s * P : e * P, :]],
                            outs=[tbl1_d[NCORE * s * P : NCORE * e * P, :]],
                        )
            # x staging is dead after the table build; release its 20KB
            xtpool_cm.__exit__(None, None, None)

            chunk_cols = []
            chunk_slot0 = []
            col = acc_slots = 0
            for ch in chunks:
                chunk_cols.append(col)
                chunk_slot0.append(acc_slots)
                d_c = sum(s[2] for s in ch)
                col += 8 * d_c
                acc_slots += d_c

            def pair_reduce(wt, soff, off, dn, TW):
                """One DVE pair-add level over wt slots [off, off+dn), cols
                [0,TW), writing pairs to scratch slots starting at soff.
                Returns list of (tile-ish AP) slot sums to feed the PE."""
                outs = []
                npair = dn // 2
                if npair:
                    nc.vector.tensor_tensor(
                        out=wt[:, soff : soff + npair, 0:TW],
                        in0=wt[:, off : off + 2 * npair - 1 : 2, 0:TW],
                        in1=wt[:, off + 1 : off + 2 * npair : 2, 0:TW],
                        op=OP.add)
                    outs = [wt[:, soff + i, 0:TW] for i in range(npair)]
                if dn % 2:
                    outs.append(wt[:, off + dn - 1, 0:TW])
                return outs

            # per band: number of PE accumulation matmuls (pairs + leftovers)
            n_mm = {}
            for ch in chunks:
                for (k, j0, dn) in ch:
                    n_mm[k] = n_mm.get(k, 0) + dn // 2 + dn % 2

            # ---------------- pass A: layer-1 windows -----------------------
            spool_cm = tc.tile_pool(name="small", bufs=8)
            spool = spool_cm.__enter__()
            gpool_cm = tc.tile_pool(name="gatA", bufs=6)
            gpool = gpool_cm.__enter__()
            wpool_cm = tc.tile_pool(name="wtA", bufs=3)
            wpool = wpool_cm.__enter__()

            def epilogue_a(k, acc):
                den = spool.tile([P, 5], f32, tag="den")
                nc.scalar.activation(den[:], acc[:, 320:325], AF.Copy,
                                     scale=float(HEADS), bias=HEADS * 1e-16)
                rec = spool.tile([P, 5], f32, tag="rec")
                nc.vector.reciprocal(rec[:], den[:])
                tmp = spool.tile([P, H1, HEADS], f32, tag="tmp1")
                nc.vector.tensor_tensor(
                    out=tmp[:],
                    in0=acc[:, 0:320].rearrange("p (c h) -> p c h", c=H1),
                    in1=rec[:].unsqueeze(1).to_broadcast([P, H1, HEADS]),
                    op=OP.mult,
                )
                o64 = spool.tile([P, H1], f32, tag="o64")
                nc.vector.tensor_reduce(out=o64[:], in_=tmp[:],
                                        axis=mybir.AxisListType.X, op=OP.add)
                o64b = spool.tile([P, H1], f32, tag="o64b")
                nc.vector.tensor_tensor(out=o64b[:], in0=o64[:], in1=b1_t[:],
                                        op=OP.add)
                nc.scalar.activation(o64[:], o64b[:], AF.Relu)
                pst = ptpool.tile([H1, P], f32, space="PSUM", tag="pst")
                nc.tensor.transpose(pst[:], o64[:], ident[:])
                nc.vector.tensor_copy(h1T[:, k * P : (k + 1) * P], pst[:])
                # layer-2/3 table rows for this band
                ps2 = pspool.tile([P, WB], f32, space="PSUM", tag="xwps")
                nc.tensor.matmul(ps2[:], h1T[:, k * P : (k + 1) * P], w2_t[:],
                                 start=True, stop=True)
                row2_t = iopool.tile([P, WB], f16, tag="rowt2")
                nc.scalar.activation(row2_t[:], ps2[:], AF.Copy)
                nc.vector.tensor_copy(ad23[:, k, :], ps2[:, 330:340])
                nc.sync.dma_start(con2_d[k * P : (k + 1) * P, :], row2_t[:])
                for (s, e) in BLOCKS2:
                    if k == e - 1:
                        nc.gpsimd.collective_compute(
                            "AllGather", mybir.AluOpType.bypass,
                            replica_groups=rg,
                            ins=[con2_d[s * P : e * P, :]],
                            outs=[tbl2_d[NCORE * s * P : NCORE * e * P, :]],
                        )

            # per-chunk: gather -> logit adds -> prelu -> exp -> fp8->f16
            # payload convert (ACT). The alpha-weighting multiply, DVE
            # pair-add level and PE accumulation matmuls run one chunk
            # behind so the DVE never stalls on the ACT round-trip.
            acc_of = {}
            mm_done = {}
            pend = None
            ready = []

            def weight_and_aggregate(ch, gt, wt, ci):
                d_tot = sum(s[2] for s in ch)
                nc.vector.tensor_tensor(
                    out=wt[:, 0:d_tot, 0:320].rearrange(
                        "p d (c h) -> p d c h", c=H1),
                    in0=wt[:, 0:d_tot, 0:320].rearrange(
                        "p d (c h) -> p d c h", c=H1),
                    in1=wt[:, 0:d_tot, 320:325].unsqueeze(2).to_broadcast(
                        [P, d_tot, H1, HEADS]),
                    op=OP.mult,
                )
                off = 0
                soff = 8
                for (k, j0, dn) in ch:
                    if k not in acc_of:
                        acc_of[k] = papool.tile([P, 336], f32, space="PSUM",
                                                name="acc", tag="acc")
                        mm_done[k] = 0
                    acc = acc_of[k]
                    segs = pair_reduce(wt, soff, off, dn, TWA)
                    soff += dn // 2
                    for seg in segs:
                        nc.tensor.matmul(acc[:, 0:TWA], ident_t[:], seg,
                                         start=(mm_done[k] == 0),
                                         stop=(mm_done[k] == n_mm[k] - 1),
                                         skip_group_check=True)
                        mm_done[k] += 1
                    if mm_done[k] == n_mm[k]:
                        ready.append((k, acc_of.pop(k)))
                    off += dn

            for ci, ch in enumerate(chunks):
                d_tot = sum(s[2] for s in ch)
                coff = chunk_cols[ci]
                gt = gpool.tile([P, SLOT_CAP, WBB], i8, tag="gt")
                nidx = P * d_tot
                nc.gpsimd.dma_gather(
                    gt[:, 0:d_tot, :], tbl1_d[:],
                    idx1_t[:, coff : coff + 8 * d_tot], nidx, nidx, WBB,
                    queue_num=ci % 4,
                )
                wt = wpool.tile([P, 12, WA], f16, tag="wt")
                ut = spool.tile([P, SLOT_CAP, 5], f16, tag="ut")
                off = 0
                for (k, j0, dn) in ch:
                    nc.vector.tensor_tensor(
                        out=ut[:, off : off + dn, :],
                        in0=gt[:, off : off + dn, 320:330].bitcast(f16),
                        in1=ad1[:, k, :].unsqueeze(1).to_broadcast([P, dn, 5]),
                        op=OP.add,
                    )
                    off += dn
                lt = spool.tile([P, SLOT_CAP, 5], f16, tag="lt")
                nc.scalar.activation(lt[:, 0:d_tot, :], ut[:, 0:d_tot, :],
                                     AF.Prelu, alpha=NEG)
                nc.scalar.activation(wt[:, 0:d_tot, 320:325],
                                     lt[:, 0:d_tot, :], AF.Exp, bias=ebias[:])
                nc.scalar.activation(wt[:, 0:d_tot, 0:320],
                                     gt[:, 0:d_tot, 0:320].bitcast(f8), AF.Copy)
                if pend is not None:
                    weight_and_aggregate(*pend)
                    for (k, acc) in ready:
                        epilogue_a(k, acc)
                    ready.clear()
                pend = (ch, gt, wt, ci)
            weight_and_aggregate(*pend)
            pend = None
            for (k, acc) in ready:
                epilogue_a(k, acc)
            ready.clear()
            wpool_cm.__exit__(None, None, None)
            gpool_cm.__exit__(None, None, None)

            # ---------------- pass B: layer-2/3 windows ---------------------
            gpool_cm = tc.tile_pool(name="gatB", bufs=8)
            gpool = gpool_cm.__enter__()
            wpool_cm = tc.tile_pool(name="wtB", bufs=3)
            wpool = wpool_cm.__enter__()

            def epilogue_b(k, acc):
                den = spool.tile([P, 10], f32, tag="den23")
                nc.scalar.activation(den[:], acc[:, 320:330], AF.Copy,
                                     scale=float(HEADS), bias=HEADS * 1e-16)
                rec = spool.tile([P, 10], f32, tag="rec23")
                nc.vector.reciprocal(rec[:], den[:])
                tmp = spool.tile([P, 2 * H2, HEADS], f32, tag="tmp2")
                nc.vector.tensor_tensor(
                    out=tmp[:].rearrange("p (l c) h -> p l c h", l=2),
                    in0=acc[:, 0:320].rearrange("p (l c h) -> p l c h",
                                                l=2, c=H2),
                    in1=rec[:].rearrange("p (l h) -> p l h", l=2).unsqueeze(2)
                    .to_broadcast([P, 2, H2, HEADS]),
                    op=OP.mult,
                )
                o64 = spool.tile([P, 2 * H2], f32, tag="o64b2")
                nc.vector.tensor_reduce(out=o64[:], in_=tmp[:],
                                        axis=mybir.AxisListType.X, op=OP.add)
                o64b = spool.tile([P, 2 * H2], f32, tag="o64c2")
                nc.vector.tensor_tensor(out=o64b[:], in0=o64[:], in1=b23_t[:],
                                        op=OP.add)
                nc.sync.dma_start(out_d[k * P : (k + 1) * P, :], o64b[:])

            acc_of = {}
            mm_done = {}
            pend = None
            ready = []

            def weight_and_aggregate_b(ch, gt, wt, ci):
                d_tot = sum(s[2] for s in ch)
                for (li, dsl) in ((0, slice(320, 325)), (1, slice(325, 330))):
                    nc.vector.tensor_tensor(
                        out=gt[:, 0:d_tot, 160 * li : 160 * li + 160].rearrange(
                            "p d (c h) -> p d c h", c=H2),
                        in0=gt[:, 0:d_tot, 160 * li : 160 * li + 160].rearrange(
                            "p d (c h) -> p d c h", c=H2),
                        in1=gt[:, 0:d_tot, dsl].unsqueeze(2).to_broadcast(
                            [P, d_tot, H2, HEADS]),
                        op=OP.mult,
                    )
                off = 0
                soff = 0
                for (k, j0, dn) in ch:
                    if k not in acc_of:
                        acc_of[k] = papool.tile([P, 336], f32, space="PSUM",
                                                name="acc", tag="acc")
                        mm_done[k] = 0
                    acc = acc_of[k]
                    npair = dn // 2
                    segs = []
                    if npair:
                        nc.vector.tensor_tensor(
                            out=wt[:, soff : soff + npair, 0:TWB],
                            in0=gt[:, off : off + 2 * npair - 1 : 2, 0:TWB],
                            in1=gt[:, off + 1 : off + 2 * npair : 2, 0:TWB],
                            op=OP.add)
                        segs = [wt[:, soff + i, 0:TWB] for i in range(npair)]
                        soff += npair
                    if dn % 2:
                        segs.append(gt[:, off + dn - 1, 0:TWB])
                    for seg in segs:
                        nc.tensor.matmul(acc[:, 0:TWB], ident_t[:], seg,
                                         start=(mm_done[k] == 0),
                                         stop=(mm_done[k] == n_mm[k] - 1),
                                         skip_group_check=True)
                        mm_done[k] += 1
                    if mm_done[k] == n_mm[k]:
                        ready.append((k, acc_of.pop(k)))
                    off += dn

            for ci, ch in enumerate(chunks):
                d_tot = sum(s[2] for s in ch)
                coff = chunk_cols[ci]
                gt = gpool.tile([P, SLOT_CAP, WB], f16, tag="gt2")
                nidx = P * d_tot
                nc.gpsimd.dma_gather(
                    gt[:, 0:d_tot, :], tbl2_d[:],
                    idx2_t[:, coff : coff + 8 * d_tot], nidx, nidx, WB,
                    queue_num=ci % 4,
                )
                wt = wpool.tile([P, 4, TWB], f16, tag="wt2")
                ut = spool.tile([P, SLOT_CAP, 10], f16, tag="ut23")
                off = 0
                for (k, j0, dn) in ch:
                    nc.vector.tensor_tensor(
                        out=ut[:, off : off + dn, :],
                        in0=gt[:, off : off + dn, 320:330],
                        in1=ad23[:, k, :].unsqueeze(1).to_broadcast([P, dn, 10]),
                        op=OP.add,
                    )
                    off += dn
                lt = spool.tile([P, SLOT_CAP, 10], f16, tag="lt23")
                nc.scalar.activation(lt[:, 0:d_tot, :], ut[:, 0:d_tot, :],
                                     AF.Prelu, alpha=NEG)
                nc.scalar.activation(gt[:, 0:d_tot, 320:330],
                                     lt[:, 0:d_tot, :], AF.Exp, bias=ebias[:])
                if pend is not None:
                    weight_and_aggregate_b(*pend)
                    for (k, acc) in ready:
                        epilogue_b(k, acc)
                    ready.clear()
                pend = (ch, gt, wt, ci)
            weight_and_aggregate_b(*pend)
            pend = None
            for (k, acc) in ready:
                epilogue_b(k, acc)
            ready.clear()
            wpool_cm.__exit__(None, None, None)
            gpool_cm.__exit__(None, None, None)
            spool_cm.__exit__(None, None, None)

    nc.compile()
    return nc


# ----------------------------------------------------------------------------
# entry point
# ----------------------------------------------------------------------------
def kernel(x, edge_index, W1, att_src1, att_dst1, b1,
           W2, att_src2, att_dst2, b2,
           W3, att_src3, att_dst3, b3):
    global _compiled
    from concourse.bass_utils import run_bass_kernel_spmd

    x = np.asarray(x, np.float32)
    edge_index = np.asarray(edge_index)

    idx1_all, idx2_all, meta = _prep_graph(edge_index.astype(np.int64))
    chunks, idx_cols = meta["chunks"], meta["idx_cols"]
    D_band = meta["D_band"]

    key = (tuple(tuple(ch) for ch in chunks), idx_cols, D_band)
    if _compiled is None or _compiled[0] != key:
        nc = _build_program(chunks, idx_cols, D_band)
        _compiled = (key, nc)
    nc = _compiled[1]

    # host-side weight augmentation (payload columns in c-major order)
    w1s, w1dst = _w_aug(np.asarray(W1, np.float32), np.asarray(att_src1),
                        np.asarray(att_dst1), HEADS, H1)
    w1big = np.zeros((FIN, WB), np.float32)
    w1big[:, 0:320] = _cmajor(np.asarray(W1, np.float32), HEADS, H1)
    w1big[:, 320:325] = w1s
    w1big[:, 325:330] = w1dst

    w2s, w2dst = _w_aug(np.asarray(W2, np.float32), np.asarray(att_src2),
                        np.asarray(att_dst2), HEADS, H2)
    w3s, w3dst = _w_aug(np.asarray(W3, np.float32), np.asarray(att_src3),
                        np.asarray(att_dst3), HEADS, H2)
    w2big = np.zeros((H1, WB), np.float32)
    w2big[:, 0:160] = _cmajor(np.asarray(W2, np.float32), HEADS, H2)
    w2big[:, 160:320] = _cmajor(np.asarray(W3, np.float32), HEADS, H2)
    w2big[:, 320:325] = w2s
    w2big[:, 325:330] = w3s
    w2big[:, 330:335] = w2dst
    w2big[:, 335:340] = w3dst

    # fp8 sentinel row (table 1): payload 0, fp16 logit halves = -1e4
    sent_row = np.zeros((1, WBB), np.int8)
    sent_row.view(np.float16)[0, 160:170] = -1e4
    # fp16 sentinel row (table 2)
    sent2_row = np.zeros((1, WB), np.float16)
    sent2_row[0, 320:340] = -1e4

    core, band, slot = meta["core"], meta["band"], meta["slot"]
    in_maps = []
    for c in range(NCORE):
        m = core == c
        xT = np.zeros((MLOC, FIN), np.float32)
        xT[band[m] * P + slot[m]] = x[m]
        in_maps.append({
            "xT": np.ascontiguousarray(xT.T).astype(ml_dtypes.bfloat16),
            "w1big": w1big.astype(ml_dtypes.bfloat16),
            "w2big": w2big, "sent": sent_row,
            "sent2": sent2_row,
            "idx1": np.ascontiguousarray(idx1_all[c]),
            "idx2": np.ascontiguousarray(idx2_all[c]),
            "b1r": np.tile(np.asarray(b1, np.float32)[None, :], (P, 1)),
            "b23r": np.tile(np.concatenate([np.asarray(b2, np.float32),
                                            np.asarray(b3, np.float32)])[None, :],
                            (P, 1)),
        })

    global LAST_RESULTS
    res = run_bass_kernel_spmd(nc, in_maps, core_ids=list(range(NCORE)),
                               trace=TRACE, tmpdir=TRACE_DIR)
    LAST_RESULTS = res

    mu = np.empty((N, H2), np.float32)
    lv = np.empty((N, H2), np.float32)
    rows = band * P + slot
    for c in range(NCORE):
        m = core == c
        o = res.results[c]["out"][rows[m]]
        mu[m] = o[:, 0:H2]
        lv[m] = o[:, H2 : 2 * H2]
    return mu, mu.copy(), lv


# revision 47
# speedup vs baseline: 1.0786x; 1.0786x over previous
"""GATModelVAE (2-layer GAT encoder VAE, eval mode) on 8 Trainium2 NeuronCores.

Strategy: destination-node (graph) parallelism. Nodes are packed into
160 windows of 128 dst nodes (degree-sorted, banded so all 8 cores run an
identical program). Per window, incoming edges live in an ELL (slot-major)
layout: slot j of partition n is the j-th in-edge of window-node n; padded
slots point at a sentinel table row whose att-logit columns are -1e4 so
exp() gives exactly 0. Per-edge source features arrive via dma_gather from
an AllGather-replicated table (payload stored c-major i.e. head-minor, and
fp8 for layer 1, converted to f16 on the scalar engine so the DVE alpha-
weighting multiply runs in its fast packed-16-bit 2x mode). The weighting
is one in-place DVE multiply per chunk; the segment sum over edge slots is
one DVE pair-add level followed by PSUM-accumulated identity matmuls (half
the matmul count of slot-at-a-time accumulation), with the exp columns
riding along to yield the softmax denominators. The table build runs in
bf16, and both tables' AllGathers are split into band-blocks scheduled to
hide behind the CC barrier (table 1) and the pass-A window tail (table 2).
Softmax normalization (constant per destination node) is applied after
aggregation in the window epilogue.
"""

import sys

sys.path.insert(0, "/opt/trn_rl_repo")

import numpy as np
import ml_dtypes

N = 20000
E0 = 320000
FIN = 256
H1 = 64
H2 = 32
HEADS = 5
NEG = 0.2

NCORE = 8
P = 128
NWIN = 160            # global windows
NB = NWIN // NCORE    # windows (bands) per core: 20
MLOC = NB * P         # node slots per core: 2560
CONTRIB = MLOC        # per-core AG contribution rows
TROWS = NCORE * MLOC + 8   # + locally-written sentinel row (pad to 8)
SENT = NCORE * MLOC   # sentinel table row
WB = 384              # matmul row width (f32 elems) for the table-build PSUM
WBB = 512             # gathered table-1 row width in BYTES (fp8 payload; %256)
SLOT_CAP = 8          # max ELL slots per gather chunk (1024 idx = 64-desc packet cap)
# exp() is stored in f16 and pair-summed; a constant bias of -ln(16) on the
# exponent scales all numerators AND denominators by 1/16 (cancels in the
# softmax) giving 16x overflow headroom in the f16 partial sums.
EXP_BIAS = -2.772588722239781
# AllGather band-blocks. The first collective can't start before the global
# CC barrier (~50us), so AG1 uses two big blocks; AG2 is front-loaded with a
# small tail so pass B isn't stuck behind a large final AllGather.
BLOCKS1 = ((0, 10), (10, 20))
BLOCKS2 = ((0, 9), (9, 15), (15, 20))

_compiled = None  # (key, nc)
TRACE = False          # set True (e.g. from test.py) to capture an NTFF profile
TRACE_DIR = None       # optional dir for trace artifacts
LAST_RESULTS = None    # BassKernelResults of the most recent run


# ----------------------------------------------------------------------------
# host-side graph preparation
# ----------------------------------------------------------------------------
def _prep_graph(edge_index):
    src = np.concatenate([edge_index[0], np.arange(N, dtype=np.int64)])
    dst = np.concatenate([edge_index[1], np.arange(N, dtype=np.int64)])
    EE = src.shape[0]
    deg = np.bincount(dst, minlength=N)

    order = np.argsort(-deg, kind="stable")      # nodes by degree desc
    pos = np.empty(N, np.int64)
    pos[order] = np.arange(N)
    win = pos // P                               # global window id
    slot = pos % P
    core = win % NCORE
    band = win // NCORE

    # slots per band = max degree in band (shared by all 8 cores)
    D_band = np.zeros(NB, np.int64)
    np.maximum.at(D_band, band, deg)
    D_band = np.maximum(D_band, 1)

    # table rows follow the blocked AG layouts of BLOCKS1 / BLOCKS2
    def blocked_rows(blocks):
        tr = np.empty(N, np.int64)
        for (s, e) in blocks:
            m = (band >= s) & (band < e)
            tr[m] = (NCORE * s * P + core[m] * (e - s) * P
                     + (band[m] - s) * P + slot[m])
        return tr

    trow1 = blocked_rows(BLOCKS1)
    trow2 = blocked_rows(BLOCKS2)

    # per-edge ELL coordinates: (core, band, slot of dst, j = rank among dst's edges)
    eorder = np.argsort(dst, kind="stable")
    ds = dst[eorder]
    run_start = np.r_[0, np.flatnonzero(ds[1:] != ds[:-1]) + 1]
    j_in = np.arange(EE) - np.repeat(run_start, np.diff(np.r_[run_start, EE]))
    es, ed = src[eorder], ds

    ec, eb, eslot = core[ed], band[ed], slot[ed]

    # global chunk layout: chunks of exactly SLOT_CAP slots, crossing band
    # boundaries; each chunk is a list of (band, j0, n_slots) segments
    chunks = []
    cur, cap = [], SLOT_CAP
    for k in range(NB):
        d, j = int(D_band[k]), 0
        while d > 0:
            t = min(cap, d)
            cur.append((k, j, t))
            j += t
            d -= t
            cap -= t
            if cap == 0:
                chunks.append(cur)
                cur, cap = [], SLOT_CAP
    if cur:
        chunks.append(cur)

    # build per-core wrapped int16 index tensors
    idx_cols = sum(8 * sum(s[2] for s in ch) for ch in chunks)

    def build_idx(trow):
        esrc_row = trow[es].astype(np.int32)
        idx_all = np.full((NCORE, 16, idx_cols), SENT, np.int16)
        ell = {}
        for k in range(NB):
            a = np.full((NCORE, int(D_band[k]), P), SENT, np.int32)
            m = eb == k
            a[ec[m], j_in[m], eslot[m]] = esrc_row[m]
            ell[k] = a
        col = 0
        for ch in chunks:
            blk = np.concatenate(
                [ell[k][:, j0 : j0 + dn, :] for (k, j0, dn) in ch], axis=1
            ).reshape(NCORE, -1)
            d_c = sum(s[2] for s in ch)
            wrapped = blk.reshape(NCORE, -1, 16).transpose(0, 2, 1)
            idx_all[:, :, col : col + 8 * d_c] = wrapped.astype(np.int16)
            col += 8 * d_c
        assert col == idx_cols
        return np.tile(idx_all, (1, 8, 1))

    meta = dict(
        chunks=chunks, idx_cols=idx_cols, core=core, band=band, slot=slot,
        D_band=tuple(int(x) for x in D_band),
    )
    return build_idx(trow1), build_idx(trow2), meta


def _w_aug(W, att_s, att_d, heads, hc):
    fin = W.shape[0]
    Wr = W.reshape(fin, heads, hc)
    ws = np.einsum("fhc,hc->fh", Wr, att_s)
    wd = np.einsum("fhc,hc->fh", Wr, att_d)
    return ws.astype(np.float32), wd.astype(np.float32)


def _cmajor(W, heads, hc):
    # [fin, heads*hc] -> columns reordered so col (c*heads + h) = W[:, h*hc + c]
    fin = W.shape[0]
    return np.ascontiguousarray(
        W.reshape(fin, heads, hc).transpose(0, 2, 1).reshape(fin, heads * hc))


# ----------------------------------------------------------------------------
# device program
# ----------------------------------------------------------------------------
def _build_program(chunks, idx_cols, D_band):
    import concourse.bass as bass
    import concourse.bacc as bacc
    import concourse.mybir as mybir
    import concourse.tile as tile
    from concourse import library_config
    from concourse.masks import make_identity

    f32 = mybir.dt.float32
    bf16 = mybir.dt.bfloat16
    f16 = mybir.dt.float16
    f8 = mybir.dt.float8e4
    i8 = mybir.dt.int8
    AF = mybir.ActivationFunctionType
    OP = mybir.AluOpType

    nc = bacc.Bacc("TRN2", target_bir_lowering=False, debug=False,
                   num_devices=NCORE, num_swdge_queues=4)

    xT_d = nc.dram_tensor("xT", [FIN, MLOC], bf16, kind="ExternalInput").ap()
    w1_d = nc.dram_tensor("w1big", [FIN, WB], bf16, kind="ExternalInput").ap()
    w2_d = nc.dram_tensor("w2big", [H1, WB], f32, kind="ExternalInput").ap()
    sent_d = nc.dram_tensor("sent", [1, WBB], i8, kind="ExternalInput").ap()
    sent2_d = nc.dram_tensor("sent2", [1, WB], f16, kind="ExternalInput").ap()
    idx1_d = nc.dram_tensor("idx1", [P, idx_cols], mybir.dt.int16,
                            kind="ExternalInput").ap()
    idx2_d = nc.dram_tensor("idx2", [P, idx_cols], mybir.dt.int16,
                            kind="ExternalInput").ap()
    b1_d = nc.dram_tensor("b1r", [P, H1], f32, kind="ExternalInput").ap()
    b2_d = nc.dram_tensor("b2r", [P, H2], f32, kind="ExternalInput").ap()
    b3_d = nc.dram_tensor("b3r", [P, H2], f32, kind="ExternalInput").ap()

    mu_d = nc.dram_tensor("mu", [MLOC, H2], f32, kind="ExternalOutput").ap()
    lv_d = nc.dram_tensor("lv", [MLOC, H2], f32, kind="ExternalOutput").ap()

    dum_i = nc.dram_tensor("dumi", [8, 32], i8).ap()
    dum_o = nc.dram_tensor("dumo", [64, 32], i8, addr_space="Shared").ap()
    con1_d = nc.dram_tensor("contrib1", [CONTRIB, WBB], i8).ap()
    con2_d = nc.dram_tensor("contrib2", [CONTRIB, WB], f16).ap()
    tbl1_d = nc.dram_tensor("tbl1", [TROWS, WBB], i8, addr_space="Shared").ap()
    tbl2_d = nc.dram_tensor("tbl2", [TROWS, WB], f16, addr_space="Shared").ap()

    rg = [list(range(NCORE))]

    S_TOT = sum(D_band)                  # total ELL slot columns (352-ish)
    slot0 = [0] * NB                     # first global slot column of band k
    for k in range(1, NB):
        slot0[k] = slot0[k - 1] + D_band[k - 1]

    WA = 328                             # wt col stride pass A (325 used)
    WB2 = 336                            # wt col stride pass B (330 used)
    TWA, TWB = 325, 330                  # tree widths

    with tile.TileContext(nc) as tc:
        with (
            tc.tile_pool(name="const", bufs=1) as cpool,
            tc.tile_pool(name="resid", bufs=1) as rpool,
            tc.tile_pool(name="io", bufs=3) as iopool,
            tc.tile_pool(name="psum", bufs=3, space="PSUM") as pspool,
            tc.tile_pool(name="psumT", bufs=1, space="PSUM") as ptpool,
            tc.tile_pool(name="psumA", bufs=4, space="PSUM") as papool,
        ):
            nc.gpsimd.load_library(library_config.mlp)
            # a tiny dummy AllGather absorbs the one-time CC barrier + DMA
            # ring ramp so the first real AllGather starts without delay
            nc.gpsimd.collective_compute(
                "AllGather", mybir.AluOpType.bypass, replica_groups=rg,
                ins=[dum_i[:]], outs=[dum_o[:]])

            ident = cpool.tile([P, P], f32)
            make_identity(nc, ident[:])
            ident_t = cpool.tile([P, P], f16)
            nc.vector.tensor_copy(ident_t[:], ident[:])
            ebias = cpool.tile([P, 1], f32)
            nc.gpsimd.memset(ebias[:], EXP_BIAS)

            w1_t = cpool.tile([P, 2, WB], bf16)
            nc.sync.dma_start(w1_t[:], w1_d[:].rearrange("(k p) n -> p k n", p=P))
            w2_t = cpool.tile([H1, WB], f32)
            nc.sync.dma_start(w2_t[:], w2_d[:])
            sent_t = cpool.tile([1, WBB], i8)
            nc.sync.dma_start(sent_t[:], sent_d[:])
            sent2_t = cpool.tile([1, WB], f16)
            nc.sync.dma_start(sent2_t[:], sent2_d[:])
            b1_t = cpool.tile([P, H1], f32)
            nc.sync.dma_start(b1_t[:], b1_d[:])
            b2_t = cpool.tile([P, H2], f32)
            nc.sync.dma_start(b2_t[:], b2_d[:])
            b3_t = cpool.tile([P, H2], f32)
            nc.sync.dma_start(b3_t[:], b3_d[:])

            idx1_t = rpool.tile([P, idx_cols], mybir.dt.int16)
            nc.scalar.dma_start(idx1_t[:], idx1_d[:])
            idx2_t = rpool.tile([P, idx_cols], mybir.dt.int16)
            nc.scalar.dma_start(idx2_t[:], idx2_d[:])
            xtpool_cm = tc.tile_pool(name="xt", bufs=1)
            xtpool = xtpool_cm.__enter__()
            xt_all = xtpool.tile([P, 2, MLOC], bf16)
            nc.sync.dma_start(xt_all[:], xT_d[:].rearrange("(k p) n -> p k n", p=P))

            ad1 = rpool.tile([P, NB, 5], f32)
            ad23 = rpool.tile([P, NB, 10], f32)
            h1T = rpool.tile([H1, MLOC], f32)

            # ---------------- pass A: layer-1 table -------------------------
            nc.sync.dma_start(tbl1_d[SENT : SENT + 1, :], sent_t[:])
            nc.sync.dma_start(tbl2_d[SENT : SENT + 1, :], sent2_t[:])
            for m in range(NB):
                ps = pspool.tile([P, WB], f32, space="PSUM", tag="xwps")
                for kk in range(2):
                    nc.tensor.matmul(ps[:], xt_all[:, kk, m * P : (m + 1) * P],
                                     w1_t[:, kk, :],
                                     start=(kk == 0), stop=(kk == 1))
                row_t = iopool.tile([P, WBB], i8, tag="rowt")
                nc.scalar.activation(row_t[:, 0:320].bitcast(f8), ps[:, 0:320],
                                     AF.Copy)
                nc.vector.tensor_copy(row_t[:, 320:330].bitcast(f16),
                                      ps[:, 320:325])
                nc.vector.tensor_copy(ad1[:, m, :], ps[:, 325:330])
                nc.sync.dma_start(con1_d[m * P : (m + 1) * P, :], row_t[:])
                for (s, e) in BLOCKS1:
                    if m == e - 1:
                        nc.gpsimd.collective_compute(
                            "AllGather", mybir.AluOpType.bypass,
                            replica_groups=rg,
                            ins=[con1_d[s * P : e * P, :]],
                            outs=[tbl1_d[NCORE * s * P : NCORE * e * P, :]],
                        )
            # x staging is dead after the table build; release its 20KB
            xtpool_cm.__exit__(None, None, None)

            chunk_cols = []
            chunk_slot0 = []
            col = acc_slots = 0
            for ch in chunks:
                chunk_cols.append(col)
                chunk_slot0.append(acc_slots)
                d_c = sum(s[2] for s in ch)
                col += 8 * d_c
                acc_slots += d_c

            def pair_reduce(wt, soff, off, dn, TW):
                """One DVE pair-add level over wt slots [off, off+dn), cols
                [0,TW), writing pairs to scratch slots starting at soff.
                Returns list of (tile-ish AP) slot sums to feed the PE."""
                outs = []
                npair = dn // 2
                if npair:
                    nc.vector.tensor_tensor(
                        out=wt[:, soff : soff + npair, 0:TW],
                        in0=wt[:, off : off + 2 * npair - 1 : 2, 0:TW],
                        in1=wt[:, off + 1 : off + 2 * npair : 2, 0:TW],
                        op=OP.add)
                    outs = [wt[:, soff + i, 0:TW] for i in range(npair)]
                if dn % 2:
                    outs.append(wt[:, off + dn - 1, 0:TW])
                return outs

            # per band: number of PE accumulation matmuls (pairs + leftovers)
            n_mm = {}
            for ch in chunks:
                for (k, j0, dn) in ch:
                    n_mm[k] = n_mm.get(k, 0) + dn // 2 + dn % 2

            # ---------------- pass A: layer-1 windows -----------------------
            spool_cm = tc.tile_pool(name="small", bufs=8)
            spool = spool_cm.__enter__()
            gpool_cm = tc.tile_pool(name="gatA", bufs=6)
            gpool = gpool_cm.__enter__()
            wpool_cm = tc.tile_pool(name="wtA", bufs=3)
            wpool = wpool_cm.__enter__()

            def epilogue_a(k, acc):
                den = spool.tile([P, 5], f32, tag="den")
                nc.scalar.activation(den[:], acc[:, 320:325], AF.Copy,
                                     scale=float(HEADS), bias=HEADS * 1e-16)
                rec = spool.tile([P, 5], f32, tag="rec")
                nc.vector.reciprocal(rec[:], den[:])
                tmp = spool.tile([P, H1, HEADS], f32, tag="tmp1")
                nc.vector.tensor_tensor(
                    out=tmp[:],
                    in0=acc[:, 0:320].rearrange("p (c h) -> p c h", c=H1),
                    in1=rec[:].unsqueeze(1).to_broadcast([P, H1, HEADS]),
                    op=OP.mult,
                )
                o64 = spool.tile([P, H1], f32, tag="o64")
                nc.vector.tensor_reduce(out=o64[:], in_=tmp[:],
                                        axis=mybir.AxisListType.X, op=OP.add)
                o64b = spool.tile([P, H1], f32, tag="o64b")
                nc.vector.tensor_tensor(out=o64b[:], in0=o64[:], in1=b1_t[:],
                                        op=OP.add)
                nc.scalar.activation(o64[:], o64b[:], AF.Relu)
                pst = ptpool.tile([H1, P], f32, space="PSUM", tag="pst")
                nc.tensor.transpose(pst[:], o64[:], ident[:])
                nc.vector.tensor_copy(h1T[:, k * P : (k + 1) * P], pst[:])
                # layer-2/3 table rows for this band
                ps2 = pspool.tile([P, WB], f32, space="PSUM", tag="xwps")
                nc.tensor.matmul(ps2[:], h1T[:, k * P : (k + 1) * P], w2_t[:],
                                 start=True, stop=True)
                row2_t = iopool.tile([P, WB], f16, tag="rowt2")
                nc.scalar.activation(row2_t[:], ps2[:], AF.Copy)
                nc.vector.tensor_copy(ad23[:, k, :], ps2[:, 330:340])
                nc.sync.dma_start(con2_d[k * P : (k + 1) * P, :], row2_t[:])
                for (s, e) in BLOCKS2:
                    if k == e - 1:
                        nc.gpsimd.collective_compute(
                            "AllGather", mybir.AluOpType.bypass,
                            replica_groups=rg,
                            ins=[con2_d[s * P : e * P, :]],
                            outs=[tbl2_d[NCORE * s * P : NCORE * e * P, :]],
                        )

            # per-chunk: gather -> logit adds -> prelu -> exp -> fp8->f16
            # payload convert (ACT). The alpha-weighting multiply, DVE
            # pair-add level and PE accumulation matmuls run one chunk
            # behind so the DVE never stalls on the ACT round-trip.
            acc_of = {}
            mm_done = {}
            pend = None
            ready = []

            def weight_and_aggregate(ch, gt, wt, ci):
                d_tot = sum(s[2] for s in ch)
                nc.vector.tensor_tensor(
                    out=wt[:, 0:d_tot, 0:320].rearrange(
                        "p d (c h) -> p d c h", c=H1),
                    in0=wt[:, 0:d_tot, 0:320].rearrange(
                        "p d (c h) -> p d c h", c=H1),
                    in1=wt[:, 0:d_tot, 320:325].unsqueeze(2).to_broadcast(
                        [P, d_tot, H1, HEADS]),
                    op=OP.mult,
                )
                off = 0
                soff = 8
                for (k, j0, dn) in ch:
                    if k not in acc_of:
                        acc_of[k] = papool.tile([P, 336], f32, space="PSUM",
                                                name="acc", tag="acc")
                        mm_done[k] = 0
                    acc = acc_of[k]
                    segs = pair_reduce(wt, soff, off, dn, TWA)
                    soff += dn // 2
                    for seg in segs:
                        nc.tensor.matmul(acc[:, 0:TWA], ident_t[:], seg,
                                         start=(mm_done[k] == 0),
                                         stop=(mm_done[k] == n_mm[k] - 1),
                                         skip_group_check=True)
                        mm_done[k] += 1
                    if mm_done[k] == n_mm[k]:
                        ready.append((k, acc_of.pop(k)))
                    off += dn

            for ci, ch in enumerate(chunks):
                d_tot = sum(s[2] for s in ch)
                coff = chunk_cols[ci]
                gt = gpool.tile([P, SLOT_CAP, WBB], i8, tag="gt")
                nidx = P * d_tot
                nc.gpsimd.dma_gather(
                    gt[:, 0:d_tot, :], tbl1_d[:],
                    idx1_t[:, coff : coff + 8 * d_tot], nidx, nidx, WBB,
                    queue_num=ci % 4,
                )
                wt = wpool.tile([P, 12, WA], f16, tag="wt")
                ut = spool.tile([P, SLOT_CAP, 5], f16, tag="ut")
                off = 0
                for (k, j0, dn) in ch:
                    nc.vector.tensor_tensor(
                        out=ut[:, off : off + dn, :],
                        in0=gt[:, off : off + dn, 320:330].bitcast(f16),
                        in1=ad1[:, k, :].unsqueeze(1).to_broadcast([P, dn, 5]),
                        op=OP.add,
                    )
                    off += dn
                lt = spool.tile([P, SLOT_CAP, 5], f16, tag="lt")
                nc.scalar.activation(lt[:, 0:d_tot, :], ut[:, 0:d_tot, :],
                                     AF.Prelu, alpha=NEG)
                nc.scalar.activation(wt[:, 0:d_tot, 320:325],
                                     lt[:, 0:d_tot, :], AF.Exp, bias=ebias[:])
                nc.scalar.activation(wt[:, 0:d_tot, 0:320],
                                     gt[:, 0:d_tot, 0:320].bitcast(f8), AF.Copy)
                if pend is not None:
                    weight_and_aggregate(*pend)
                    for (k, acc) in ready:
                        epilogue_a(k, acc)
                    ready.clear()
                pend = (ch, gt, wt, ci)
            weight_and_aggregate(*pend)
            pend = None
            for (k, acc) in ready:
                epilogue_a(k, acc)
            ready.clear()
            wpool_cm.__exit__(None, None, None)
            gpool_cm.__exit__(None, None, None)

            # ---------------- pass B: layer-2/3 windows ---------------------
            gpool_cm = tc.tile_pool(name="gatB", bufs=8)
            gpool = gpool_cm.__enter__()
            wpool_cm = tc.tile_pool(name="wtB", bufs=3)
            wpool = wpool_cm.__enter__()

            def epilogue_b(k, acc):
                den = spool.tile([P, 10], f32, tag="den23")
                nc.scalar.activation(den[:], acc[:, 320:330], AF.Copy,
                                     scale=float(HEADS), bias=HEADS * 1e-16)
                rec = spool.tile([P, 10], f32, tag="rec23")
                nc.vector.reciprocal(rec[:], den[:])
                for (li, xsl, b_t, out_d) in (
                    (0, slice(0, 160), b2_t, mu_d),
                    (1, slice(160, 320), b3_t, lv_d),
                ):
                    tmp = spool.tile([P, H2, HEADS], f32, tag="tmp2")
                    nc.vector.tensor_tensor(
                        out=tmp[:],
                        in0=acc[:, xsl].rearrange("p (c h) -> p c h", c=H2),
                        in1=rec[:, 5 * li : 5 * li + 5].unsqueeze(1)
                        .to_broadcast([P, H2, HEADS]),
                        op=OP.mult,
                    )
                    o32 = spool.tile([P, H2], f32, tag="o32")
                    nc.vector.tensor_reduce(out=o32[:], in_=tmp[:],
                                            axis=mybir.AxisListType.X, op=OP.add)
                    o32b = spool.tile([P, H2], f32, tag="o32b")
                    nc.vector.tensor_tensor(out=o32b[:], in0=o32[:], in1=b_t[:],
                                            op=OP.add)
                    nc.sync.dma_start(out_d[k * P : (k + 1) * P, :], o32b[:])

            acc_of = {}
            mm_done = {}
            pend = None
            ready = []

            def weight_and_aggregate_b(ch, gt, wt, ci):
                d_tot = sum(s[2] for s in ch)
                for (li, dsl) in ((0, slice(320, 325)), (1, slice(325, 330))):
                    nc.vector.tensor_tensor(
                        out=gt[:, 0:d_tot, 160 * li : 160 * li + 160].rearrange(
                            "p d (c h) -> p d c h", c=H2),
                        in0=gt[:, 0:d_tot, 160 * li : 160 * li + 160].rearrange(
                            "p d (c h) -> p d c h", c=H2),
                        in1=gt[:, 0:d_tot, dsl].unsqueeze(2).to_broadcast(
                            [P, d_tot, H2, HEADS]),
                        op=OP.mult,
                    )
                off = 0
                soff = 0
                for (k, j0, dn) in ch:
                    if k not in acc_of:
                        acc_of[k] = papool.tile([P, 336], f32, space="PSUM",
                                                name="acc", tag="acc")
                        mm_done[k] = 0
                    acc = acc_of[k]
                    npair = dn // 2
                    segs = []
                    if npair:
                        nc.vector.tensor_tensor(
                            out=wt[:, soff : soff + npair, 0:TWB],
                            in0=gt[:, off : off + 2 * npair - 1 : 2, 0:TWB],
                            in1=gt[:, off + 1 : off + 2 * npair : 2, 0:TWB],
                            op=OP.add)
                        segs = [wt[:, soff + i, 0:TWB] for i in range(npair)]
                        soff += npair
                    if dn % 2:
                        segs.append(gt[:, off + dn - 1, 0:TWB])
                    for seg in segs:
                        nc.tensor.matmul(acc[:, 0:TWB], ident_t[:], seg,
                                         start=(mm_done[k] == 0),
                                         stop=(mm_done[k] == n_mm[k] - 1),
                                         skip_group_check=True)
                        mm_done[k] += 1
                    if mm_done[k] == n_mm[k]:
                        ready.append((k, acc_of.pop(k)))
                    off += dn

            for ci, ch in enumerate(chunks):
                d_tot = sum(s[2] for s in ch)
                coff = chunk_cols[ci]
                gt = gpool.tile([P, SLOT_CAP, WB], f16, tag="gt2")
                nidx = P * d_tot
                nc.gpsimd.dma_gather(
                    gt[:, 0:d_tot, :], tbl2_d[:],
                    idx2_t[:, coff : coff + 8 * d_tot], nidx, nidx, WB,
                    queue_num=ci % 4,
                )
                wt = wpool.tile([P, 4, TWB], f16, tag="wt2")
                ut = spool.tile([P, SLOT_CAP, 10], f16, tag="ut23")
                off = 0
                for (k, j0, dn) in ch:
                    nc.vector.tensor_tensor(
                        out=ut[:, off : off + dn, :],
                        in0=gt[:, off : off + dn, 320:330],
                        in1=ad23[:, k, :].unsqueeze(1).to_broadcast([P, dn, 10]),
                        op=OP.add,
                    )
                    off += dn
                lt = spool.tile([P, SLOT_CAP, 10], f16, tag="lt23")
                nc.scalar.activation(lt[:, 0:d_tot, :], ut[:, 0:d_tot, :],
                                     AF.Prelu, alpha=NEG)
                nc.scalar.activation(gt[:, 0:d_tot, 320:330],
                                     lt[:, 0:d_tot, :], AF.Exp, bias=ebias[:])
                if pend is not None:
                    weight_and_aggregate_b(*pend)
                    for (k, acc) in ready:
                        epilogue_b(k, acc)
                    ready.clear()
                pend = (ch, gt, wt, ci)
            weight_and_aggregate_b(*pend)
            pend = None
            for (k, acc) in ready:
                epilogue_b(k, acc)
            ready.clear()
            wpool_cm.__exit__(None, None, None)
            gpool_cm.__exit__(None, None, None)
            spool_cm.__exit__(None, None, None)

    nc.compile()
    return nc


# ----------------------------------------------------------------------------
# entry point
# ----------------------------------------------------------------------------
def kernel(x, edge_index, W1, att_src1, att_dst1, b1,
           W2, att_src2, att_dst2, b2,
           W3, att_src3, att_dst3, b3):
    global _compiled
    from concourse.bass_utils import run_bass_kernel_spmd

    x = np.asarray(x, np.float32)
    edge_index = np.asarray(edge_index)

    idx1_all, idx2_all, meta = _prep_graph(edge_index.astype(np.int64))
    chunks, idx_cols = meta["chunks"], meta["idx_cols"]
    D_band = meta["D_band"]

    key = (tuple(tuple(ch) for ch in chunks), idx_cols, D_band)
    if _compiled is None or _compiled[0] != key:
        nc = _build_program(chunks, idx_cols, D_band)
        _compiled = (key, nc)
    nc = _compiled[1]

    # host-side weight augmentation (payload columns in c-major order)
    w1s, w1dst = _w_aug(np.asarray(W1, np.float32), np.asarray(att_src1),
                        np.asarray(att_dst1), HEADS, H1)
    w1big = np.zeros((FIN, WB), np.float32)
    w1big[:, 0:320] = _cmajor(np.asarray(W1, np.float32), HEADS, H1)
    w1big[:, 320:325] = w1s
    w1big[:, 325:330] = w1dst

    w2s, w2dst = _w_aug(np.asarray(W2, np.float32), np.asarray(att_src2),
                        np.asarray(att_dst2), HEADS, H2)
    w3s, w3dst = _w_aug(np.asarray(W3, np.float32), np.asarray(att_src3),
                        np.asarray(att_dst3), HEADS, H2)
    w2big = np.zeros((H1, WB), np.float32)
    w2big[:, 0:160] = _cmajor(np.asarray(W2, np.float32), HEADS, H2)
    w2big[:, 160:320] = _cmajor(np.asarray(W3, np.float32), HEADS, H2)
    w2big[:, 320:325] = w2s
    w2big[:, 325:330] = w3s
    w2big[:, 330:335] = w2dst
    w2big[:, 335:340] = w3dst

    # fp8 sentinel row (table 1): payload 0, fp16 logit halves = -1e4
    sent_row = np.zeros((1, WBB), np.int8)
    sent_row.view(np.float16)[0, 160:170] = -1e4
    # fp16 sentinel row (table 2)
    sent2_row = np.zeros((1, WB), np.float16)
    sent2_row[0, 320:340] = -1e4

    core, band, slot = meta["core"], meta["band"], meta["slot"]
    in_maps = []
    for c in range(NCORE):
        m = core == c
        xT = np.zeros((MLOC, FIN), np.float32)
        xT[band[m] * P + slot[m]] = x[m]
        in_maps.append({
            "xT": np.ascontiguousarray(xT.T).astype(ml_dtypes.bfloat16),
            "w1big": w1big.astype(ml_dtypes.bfloat16),
            "w2big": w2big, "sent": sent_row,
            "sent2": sent2_row,
            "idx1": np.ascontiguousarray(idx1_all[c]),
            "idx2": np.ascontiguousarray(idx2_all[c]),
            "b1r": np.tile(np.asarray(b1, np.float32)[None, :], (P, 1)),
            "b2r": np.tile(np.asarray(b2, np.float32)[None, :], (P, 1)),
            "b3r": np.tile(np.asarray(b3, np.float32)[None, :], (P, 1)),
        })

    global LAST_RESULTS
    res = run_bass_kernel_spmd(nc, in_maps, core_ids=list(range(NCORE)),
                               trace=TRACE, tmpdir=TRACE_DIR)
    LAST_RESULTS = res

    mu = np.empty((N, H2), np.float32)
    lv = np.empty((N, H2), np.float32)
    rows = band * P + slot
    for c in range(NCORE):
        m = core == c
        mu[m] = res.results[c]["mu"][rows[m]]
        lv[m] = res.results[c]["lv"][rows[m]]
    return mu, mu.copy(), lv


# revision 48
# speedup vs baseline: 1.0837x; 1.0047x over previous
"""GATModelVAE (2-layer GAT encoder VAE, eval mode) on 8 Trainium2 NeuronCores.

Strategy: destination-node (graph) parallelism. Nodes are packed into
160 windows of 128 dst nodes (degree-sorted, banded so all 8 cores run an
identical program). Per window, incoming edges live in an ELL (slot-major)
layout: slot j of partition n is the j-th in-edge of window-node n; padded
slots point at a sentinel table row whose att-logit columns are -1e4 so
exp() gives exactly 0. Per-edge source features arrive via dma_gather from
an AllGather-replicated table (payload stored c-major i.e. head-minor, and
fp8 for layer 1, converted to f16 on the scalar engine so the DVE alpha-
weighting multiply runs in its fast packed-16-bit 2x mode). The weighting
is one in-place DVE multiply per chunk; the segment sum over edge slots is
one DVE pair-add level followed by PSUM-accumulated identity matmuls (half
the matmul count of slot-at-a-time accumulation), with the exp columns
riding along to yield the softmax denominators. The table build runs in
bf16, and both tables' AllGathers are split into band-blocks scheduled to
hide behind the CC barrier (table 1) and the pass-A window tail (table 2).
Softmax normalization (constant per destination node) is applied after
aggregation in the window epilogue.
"""

import sys

sys.path.insert(0, "/opt/trn_rl_repo")

import numpy as np
import ml_dtypes

N = 20000
E0 = 320000
FIN = 256
H1 = 64
H2 = 32
HEADS = 5
NEG = 0.2

NCORE = 8
P = 128
NWIN = 160            # global windows
NB = NWIN // NCORE    # windows (bands) per core: 20
MLOC = NB * P         # node slots per core: 2560
CONTRIB = MLOC        # per-core AG contribution rows
TROWS = NCORE * MLOC + 8   # + locally-written sentinel row (pad to 8)
SENT = NCORE * MLOC   # sentinel table row
WB = 384              # matmul row width (f32 elems) for the table-build PSUM
WBB = 512             # gathered table-1 row width in BYTES (fp8 payload; %256)
SLOT_CAP = 8          # max ELL slots per gather chunk (1024 idx = 64-desc packet cap)
# exp() is stored in f16 and pair-summed; a constant bias of -ln(16) on the
# exponent scales all numerators AND denominators by 1/16 (cancels in the
# softmax) giving 16x overflow headroom in the f16 partial sums.
EXP_BIAS = -2.772588722239781
# AllGather band-blocks. The first collective can't start before the global
# CC barrier (~50us), so AG1 uses two big blocks; AG2 is front-loaded with a
# small tail so pass B isn't stuck behind a large final AllGather.
BLOCKS1 = ((0, 10), (10, 20))
BLOCKS2 = ((0, 9), (9, 15), (15, 20))

_compiled = None  # (key, nc)
TRACE = False          # set True (e.g. from test.py) to capture an NTFF profile
TRACE_DIR = None       # optional dir for trace artifacts
LAST_RESULTS = None    # BassKernelResults of the most recent run


# ----------------------------------------------------------------------------
# host-side graph preparation
# ----------------------------------------------------------------------------
def _prep_graph(edge_index):
    src = np.concatenate([edge_index[0], np.arange(N, dtype=np.int64)])
    dst = np.concatenate([edge_index[1], np.arange(N, dtype=np.int64)])
    EE = src.shape[0]
    deg = np.bincount(dst, minlength=N)

    order = np.argsort(-deg, kind="stable")      # nodes by degree desc
    pos = np.empty(N, np.int64)
    pos[order] = np.arange(N)
    win = pos // P                               # global window id
    slot = pos % P
    core = win % NCORE
    band = win // NCORE

    # slots per band = max degree in band (shared by all 8 cores)
    D_band = np.zeros(NB, np.int64)
    np.maximum.at(D_band, band, deg)
    D_band = np.maximum(D_band, 1)

    # table rows follow the blocked AG layouts of BLOCKS1 / BLOCKS2
    def blocked_rows(blocks):
        tr = np.empty(N, np.int64)
        for (s, e) in blocks:
            m = (band >= s) & (band < e)
            tr[m] = (NCORE * s * P + core[m] * (e - s) * P
                     + (band[m] - s) * P + slot[m])
        return tr

    trow1 = blocked_rows(BLOCKS1)
    trow2 = blocked_rows(BLOCKS2)

    # per-edge ELL coordinates: (core, band, slot of dst, j = rank among dst's edges)
    eorder = np.argsort(dst, kind="stable")
    ds = dst[eorder]
    run_start = np.r_[0, np.flatnonzero(ds[1:] != ds[:-1]) + 1]
    j_in = np.arange(EE) - np.repeat(run_start, np.diff(np.r_[run_start, EE]))
    es, ed = src[eorder], ds

    ec, eb, eslot = core[ed], band[ed], slot[ed]

    # global chunk layout: chunks of exactly SLOT_CAP slots, crossing band
    # boundaries; each chunk is a list of (band, j0, n_slots) segments
    chunks = []
    cur, cap = [], SLOT_CAP
    for k in range(NB):
        d, j = int(D_band[k]), 0
        while d > 0:
            t = min(cap, d)
            cur.append((k, j, t))
            j += t
            d -= t
            cap -= t
            if cap == 0:
                chunks.append(cur)
                cur, cap = [], SLOT_CAP
    if cur:
        chunks.append(cur)

    # build per-core wrapped int16 index tensors
    idx_cols = sum(8 * sum(s[2] for s in ch) for ch in chunks)

    def build_idx(trow):
        esrc_row = trow[es].astype(np.int32)
        idx_all = np.full((NCORE, 16, idx_cols), SENT, np.int16)
        ell = {}
        for k in range(NB):
            a = np.full((NCORE, int(D_band[k]), P), SENT, np.int32)
            m = eb == k
            a[ec[m], j_in[m], eslot[m]] = esrc_row[m]
            ell[k] = a
        col = 0
        for ch in chunks:
            blk = np.concatenate(
                [ell[k][:, j0 : j0 + dn, :] for (k, j0, dn) in ch], axis=1
            ).reshape(NCORE, -1)
            d_c = sum(s[2] for s in ch)
            wrapped = blk.reshape(NCORE, -1, 16).transpose(0, 2, 1)
            idx_all[:, :, col : col + 8 * d_c] = wrapped.astype(np.int16)
            col += 8 * d_c
        assert col == idx_cols
        return np.tile(idx_all, (1, 8, 1))

    meta = dict(
        chunks=chunks, idx_cols=idx_cols, core=core, band=band, slot=slot,
        D_band=tuple(int(x) for x in D_band),
    )
    return build_idx(trow1), build_idx(trow2), meta


def _w_aug(W, att_s, att_d, heads, hc):
    fin = W.shape[0]
    Wr = W.reshape(fin, heads, hc)
    ws = np.einsum("fhc,hc->fh", Wr, att_s)
    wd = np.einsum("fhc,hc->fh", Wr, att_d)
    return ws.astype(np.float32), wd.astype(np.float32)


def _cmajor(W, heads, hc):
    # [fin, heads*hc] -> columns reordered so col (c*heads + h) = W[:, h*hc + c]
    fin = W.shape[0]
    return np.ascontiguousarray(
        W.reshape(fin, heads, hc).transpose(0, 2, 1).reshape(fin, heads * hc))


# ----------------------------------------------------------------------------
# device program
# ----------------------------------------------------------------------------
def _build_program(chunks, idx_cols, D_band):
    import concourse.bass as bass
    import concourse.bacc as bacc
    import concourse.mybir as mybir
    import concourse.tile as tile
    from concourse import library_config
    from concourse.masks import make_identity

    f32 = mybir.dt.float32
    bf16 = mybir.dt.bfloat16
    f16 = mybir.dt.float16
    f8 = mybir.dt.float8e4
    i8 = mybir.dt.int8
    AF = mybir.ActivationFunctionType
    OP = mybir.AluOpType

    nc = bacc.Bacc("TRN2", target_bir_lowering=False, debug=False,
                   num_devices=NCORE, num_swdge_queues=4)

    xT_d = nc.dram_tensor("xT", [FIN, MLOC], bf16, kind="ExternalInput").ap()
    w1_d = nc.dram_tensor("w1big", [FIN, WB], bf16, kind="ExternalInput").ap()
    w2_d = nc.dram_tensor("w2big", [H1, WB], f32, kind="ExternalInput").ap()
    sent_d = nc.dram_tensor("sent", [1, WBB], i8, kind="ExternalInput").ap()
    sent2_d = nc.dram_tensor("sent2", [1, WB], f16, kind="ExternalInput").ap()
    idx1_d = nc.dram_tensor("idx1", [P, idx_cols], mybir.dt.int16,
                            kind="ExternalInput").ap()
    idx2_d = nc.dram_tensor("idx2", [P, idx_cols], mybir.dt.int16,
                            kind="ExternalInput").ap()
    b1_d = nc.dram_tensor("b1r", [P, H1], f32, kind="ExternalInput").ap()
    b23_d = nc.dram_tensor("b23r", [P, 2 * H2], f32, kind="ExternalInput").ap()

    out_d = nc.dram_tensor("out", [MLOC, 2 * H2], f32,
                           kind="ExternalOutput").ap()

    dum_i = nc.dram_tensor("dumi", [8, 32], i8).ap()
    dum_o = nc.dram_tensor("dumo", [64, 32], i8, addr_space="Shared").ap()
    con1_d = nc.dram_tensor("contrib1", [CONTRIB, WBB], i8).ap()
    con2_d = nc.dram_tensor("contrib2", [CONTRIB, WB], f16).ap()
    tbl1_d = nc.dram_tensor("tbl1", [TROWS, WBB], i8, addr_space="Shared").ap()
    tbl2_d = nc.dram_tensor("tbl2", [TROWS, WB], f16, addr_space="Shared").ap()

    rg = [list(range(NCORE))]

    S_TOT = sum(D_band)                  # total ELL slot columns (352-ish)
    slot0 = [0] * NB                     # first global slot column of band k
    for k in range(1, NB):
        slot0[k] = slot0[k - 1] + D_band[k - 1]

    WA = 328                             # wt col stride pass A (325 used)
    WB2 = 336                            # wt col stride pass B (330 used)
    TWA, TWB = 325, 330                  # tree widths

    with tile.TileContext(nc) as tc:
        with (
            tc.tile_pool(name="const", bufs=1) as cpool,
            tc.tile_pool(name="resid", bufs=1) as rpool,
            tc.tile_pool(name="io", bufs=3) as iopool,
            tc.tile_pool(name="psum", bufs=3, space="PSUM") as pspool,
            tc.tile_pool(name="psumT", bufs=1, space="PSUM") as ptpool,
            tc.tile_pool(name="psumA", bufs=4, space="PSUM") as papool,
        ):
            nc.gpsimd.load_library(library_config.mlp)
            # a tiny dummy AllGather absorbs the one-time CC barrier + DMA
            # ring ramp so the first real AllGather starts without delay
            nc.gpsimd.collective_compute(
                "AllGather", mybir.AluOpType.bypass, replica_groups=rg,
                ins=[dum_i[:]], outs=[dum_o[:]])

            ident = cpool.tile([P, P], f32)
            make_identity(nc, ident[:])
            ident_t = cpool.tile([P, P], f16)
            nc.vector.tensor_copy(ident_t[:], ident[:])
            ebias = cpool.tile([P, 1], f32)
            nc.gpsimd.memset(ebias[:], EXP_BIAS)

            w1_t = cpool.tile([P, 2, WB], bf16)
            nc.sync.dma_start(w1_t[:], w1_d[:].rearrange("(k p) n -> p k n", p=P))
            w2_t = cpool.tile([H1, WB], f32)
            nc.sync.dma_start(w2_t[:], w2_d[:])
            sent_t = cpool.tile([1, WBB], i8)
            nc.sync.dma_start(sent_t[:], sent_d[:])
            sent2_t = cpool.tile([1, WB], f16)
            nc.sync.dma_start(sent2_t[:], sent2_d[:])
            b1_t = cpool.tile([P, H1], f32)
            nc.sync.dma_start(b1_t[:], b1_d[:])
            b23_t = cpool.tile([P, 2 * H2], f32)
            nc.sync.dma_start(b23_t[:], b23_d[:])

            idx1_t = rpool.tile([P, idx_cols], mybir.dt.int16)
            nc.scalar.dma_start(idx1_t[:], idx1_d[:])
            idx2_t = rpool.tile([P, idx_cols], mybir.dt.int16)
            nc.scalar.dma_start(idx2_t[:], idx2_d[:])
            xtpool_cm = tc.tile_pool(name="xt", bufs=1)
            xtpool = xtpool_cm.__enter__()
            xt_all = xtpool.tile([P, 2, MLOC], bf16)
            nc.sync.dma_start(xt_all[:], xT_d[:].rearrange("(k p) n -> p k n", p=P))

            ad1 = rpool.tile([P, NB, 5], f32)
            ad23 = rpool.tile([P, NB, 10], f32)
            h1T = rpool.tile([H1, MLOC], f32)

            # ---------------- pass A: layer-1 table -------------------------
            nc.sync.dma_start(tbl1_d[SENT : SENT + 1, :], sent_t[:])
            nc.sync.dma_start(tbl2_d[SENT : SENT + 1, :], sent2_t[:])
            for m in range(NB):
                ps = pspool.tile([P, WB], f32, space="PSUM", tag="xwps")
                for kk in range(2):
                    nc.tensor.matmul(ps[:], xt_all[:, kk, m * P : (m + 1) * P],
                                     w1_t[:, kk, :],
                                     start=(kk == 0), stop=(kk == 1))
                row_t = iopool.tile([P, WBB], i8, tag="rowt")
                nc.scalar.activation(row_t[:, 0:320].bitcast(f8), ps[:, 0:320],
                                     AF.Copy)
                nc.vector.tensor_copy(row_t[:, 320:330].bitcast(f16),
                                      ps[:, 320:325])
                nc.vector.tensor_copy(ad1[:, m, :], ps[:, 325:330])
                nc.sync.dma_start(con1_d[m * P : (m + 1) * P, :], row_t[:])
                for (s, e) in BLOCKS1:
                    if m == e - 1:
                        nc.gpsimd.collective_compute(
                            "AllGather", mybir.AluOpType.bypass,
                            replica_groups=rg,
                            ins=[con1_d[s * P : e * P, :]],
                            outs=[tbl1_d[NCORE * s * P : NCORE * e * P, :]],
                        )
            # x staging is dead after the table build; release its 20KB
            xtpool_cm.__exit__(None, None, None)

            chunk_cols = []
            chunk_slot0 = []
            col = acc_slots = 0
            for ch in chunks:
                chunk_cols.append(col)
                chunk_slot0.append(acc_slots)
                d_c = sum(s[2] for s in ch)
                col += 8 * d_c
                acc_slots += d_c

            def pair_reduce(wt, soff, off, dn, TW):
                """One DVE pair-add level over wt slots [off, off+dn), cols
                [0,TW), writing pairs to scratch slots starting at soff.
                Returns list of (tile-ish AP) slot sums to feed the PE."""
                outs = []
                npair = dn // 2
                if npair:
                    nc.vector.tensor_tensor(
                        out=wt[:, soff : soff + npair, 0:TW],
                        in0=wt[:, off : off + 2 * npair - 1 : 2, 0:TW],
                        in1=wt[:, off + 1 : off + 2 * npair : 2, 0:TW],
                        op=OP.add)
                    outs = [wt[:, soff + i, 0:TW] for i in range(npair)]
                if dn % 2:
                    outs.append(wt[:, off + dn - 1, 0:TW])
                return outs

            # per band: number of PE accumulation matmuls (pairs + leftovers)
            n_mm = {}
            for ch in chunks:
                for (k, j0, dn) in ch:
                    n_mm[k] = n_mm.get(k, 0) + dn // 2 + dn % 2

            # ---------------- pass A: layer-1 windows -----------------------
            spool_cm = tc.tile_pool(name="small", bufs=8)
            spool = spool_cm.__enter__()
            gpool_cm = tc.tile_pool(name="gatA", bufs=6)
            gpool = gpool_cm.__enter__()
            wpool_cm = tc.tile_pool(name="wtA", bufs=3)
            wpool = wpool_cm.__enter__()

            def epilogue_a(k, acc):
                den = spool.tile([P, 5], f32, tag="den")
                nc.scalar.activation(den[:], acc[:, 320:325], AF.Copy,
                                     scale=float(HEADS), bias=HEADS * 1e-16)
                rec = spool.tile([P, 5], f32, tag="rec")
                nc.vector.reciprocal(rec[:], den[:])
                tmp = spool.tile([P, H1, HEADS], f32, tag="tmp1")
                nc.vector.tensor_tensor(
                    out=tmp[:],
                    in0=acc[:, 0:320].rearrange("p (c h) -> p c h", c=H1),
                    in1=rec[:].unsqueeze(1).to_broadcast([P, H1, HEADS]),
                    op=OP.mult,
                )
                o64 = spool.tile([P, H1], f32, tag="o64")
                nc.vector.tensor_reduce(out=o64[:], in_=tmp[:],
                                        axis=mybir.AxisListType.X, op=OP.add)
                o64b = spool.tile([P, H1], f32, tag="o64b")
                nc.vector.tensor_tensor(out=o64b[:], in0=o64[:], in1=b1_t[:],
                                        op=OP.add)
                nc.scalar.activation(o64[:], o64b[:], AF.Relu)
                pst = ptpool.tile([H1, P], f32, space="PSUM", tag="pst")
                nc.tensor.transpose(pst[:], o64[:], ident[:])
                nc.vector.tensor_copy(h1T[:, k * P : (k + 1) * P], pst[:])
                # layer-2/3 table rows for this band
                ps2 = pspool.tile([P, WB], f32, space="PSUM", tag="xwps")
                nc.tensor.matmul(ps2[:], h1T[:, k * P : (k + 1) * P], w2_t[:],
                                 start=True, stop=True)
                row2_t = iopool.tile([P, WB], f16, tag="rowt2")
                nc.scalar.activation(row2_t[:], ps2[:], AF.Copy)
                nc.vector.tensor_copy(ad23[:, k, :], ps2[:, 330:340])
                nc.sync.dma_start(con2_d[k * P : (k + 1) * P, :], row2_t[:])
                for (s, e) in BLOCKS2:
                    if k == e - 1:
                        nc.gpsimd.collective_compute(
                            "AllGather", mybir.AluOpType.bypass,
                            replica_groups=rg,
                            ins=[con2_d[s * P : e * P, :]],
                            outs=[tbl2_d[NCORE * s * P : NCORE * e * P, :]],
                        )

            # per-chunk: gather -> logit adds -> prelu -> exp -> fp8->f16
            # payload convert (ACT). The alpha-weighting multiply, DVE
            # pair-add level and PE accumulation matmuls run one chunk
            # behind so the DVE never stalls on the ACT round-trip.
            acc_of = {}
            mm_done = {}
            pend = None
            ready = []

            def weight_and_aggregate(ch, gt, wt, ci):
                d_tot = sum(s[2] for s in ch)
                nc.vector.tensor_tensor(
                    out=wt[:, 0:d_tot, 0:320].rearrange(
                        "p d (c h) -> p d c h", c=H1),
                    in0=wt[:, 0:d_tot, 0:320].rearrange(
                        "p d (c h) -> p d c h", c=H1),
                    in1=wt[:, 0:d_tot, 320:325].unsqueeze(2).to_broadcast(
                        [P, d_tot, H1, HEADS]),
                    op=OP.mult,
                )
                off = 0
                soff = 8
                for (k, j0, dn) in ch:
                    if k not in acc_of:
                        acc_of[k] = papool.tile([P, 336], f32, space="PSUM",
                                                name="acc", tag="acc")
                        mm_done[k] = 0
                    acc = acc_of[k]
                    segs = pair_reduce(wt, soff, off, dn, TWA)
                    soff += dn // 2
                    for seg in segs:
                        nc.tensor.matmul(acc[:, 0:TWA], ident_t[:], seg,
                                         start=(mm_done[k] == 0),
                                         stop=(mm_done[k] == n_mm[k] - 1),
                                         skip_group_check=True)
                        mm_done[k] += 1
                    if mm_done[k] == n_mm[k]:
                        ready.append((k, acc_of.pop(k)))
                    off += dn

            for ci, ch in enumerate(chunks):
                d_tot = sum(s[2] for s in ch)
                coff = chunk_cols[ci]
                gt = gpool.tile([P, SLOT_CAP, WBB], i8, tag="gt")
                nidx = P * d_tot
                nc.gpsimd.dma_gather(
                    gt[:, 0:d_tot, :], tbl1_d[:],
                    idx1_t[:, coff : coff + 8 * d_tot], nidx, nidx, WBB,
                    queue_num=ci % 4,
                )
                wt = wpool.tile([P, 12, WA], f16, tag="wt")
                ut = spool.tile([P, SLOT_CAP, 5], f16, tag="ut")
                off = 0
                for (k, j0, dn) in ch:
                    nc.vector.tensor_tensor(
                        out=ut[:, off : off + dn, :],
                        in0=gt[:, off : off + dn, 320:330].bitcast(f16),
                        in1=ad1[:, k, :].unsqueeze(1).to_broadcast([P, dn, 5]),
                        op=OP.add,
                    )
                    off += dn
                lt = spool.tile([P, SLOT_CAP, 5], f16, tag="lt")
                nc.scalar.activation(lt[:, 0:d_tot, :], ut[:, 0:d_tot, :],
                                     AF.Prelu, alpha=NEG)
                nc.scalar.activation(wt[:, 0:d_tot, 320:325],
                                     lt[:, 0:d_tot, :], AF.Exp, bias=ebias[:])
                nc.scalar.activation(wt[:, 0:d_tot, 0:320],
                                     gt[:, 0:d_tot, 0:320].bitcast(f8), AF.Copy)
                if pend is not None:
                    weight_and_aggregate(*pend)
                    for (k, acc) in ready:
                        epilogue_a(k, acc)
                    ready.clear()
                pend = (ch, gt, wt, ci)
            weight_and_aggregate(*pend)
            pend = None
            for (k, acc) in ready:
                epilogue_a(k, acc)
            ready.clear()
            wpool_cm.__exit__(None, None, None)
            gpool_cm.__exit__(None, None, None)

            # ---------------- pass B: layer-2/3 windows ---------------------
            gpool_cm = tc.tile_pool(name="gatB", bufs=8)
            gpool = gpool_cm.__enter__()
            wpool_cm = tc.tile_pool(name="wtB", bufs=3)
            wpool = wpool_cm.__enter__()

            def epilogue_b(k, acc):
                den = spool.tile([P, 10], f32, tag="den23")
                nc.scalar.activation(den[:], acc[:, 320:330], AF.Copy,
                                     scale=float(HEADS), bias=HEADS * 1e-16)
                rec = spool.tile([P, 10], f32, tag="rec23")
                nc.vector.reciprocal(rec[:], den[:])
                tmp = spool.tile([P, 2 * H2, HEADS], f32, tag="tmp2")
                nc.vector.tensor_tensor(
                    out=tmp[:].rearrange("p (l c) h -> p l c h", l=2),
                    in0=acc[:, 0:320].rearrange("p (l c h) -> p l c h",
                                                l=2, c=H2),
                    in1=rec[:].rearrange("p (l h) -> p l h", l=2).unsqueeze(2)
                    .to_broadcast([P, 2, H2, HEADS]),
                    op=OP.mult,
                )
                o64 = spool.tile([P, 2 * H2], f32, tag="o64b2")
                nc.vector.tensor_reduce(out=o64[:], in_=tmp[:],
                                        axis=mybir.AxisListType.X, op=OP.add)
                o64b = spool.tile([P, 2 * H2], f32, tag="o64c2")
                nc.vector.tensor_tensor(out=o64b[:], in0=o64[:], in1=b23_t[:],
                                        op=OP.add)
                nc.sync.dma_start(out_d[k * P : (k + 1) * P, :], o64b[:])

            acc_of = {}
            mm_done = {}
            pend = None
            ready = []

            def weight_and_aggregate_b(ch, gt, wt, ci):
                d_tot = sum(s[2] for s in ch)
                for (li, dsl) in ((0, slice(320, 325)), (1, slice(325, 330))):
                    nc.vector.tensor_tensor(
                        out=gt[:, 0:d_tot, 160 * li : 160 * li + 160].rearrange(
                            "p d (c h) -> p d c h", c=H2),
                        in0=gt[:, 0:d_tot, 160 * li : 160 * li + 160].rearrange(
                            "p d (c h) -> p d c h", c=H2),
                        in1=gt[:, 0:d_tot, dsl].unsqueeze(2).to_broadcast(
                            [P, d_tot, H2, HEADS]),
                        op=OP.mult,
                    )
                off = 0
                soff = 0
                for (k, j0, dn) in ch:
                    if k not in acc_of:
                        acc_of[k] = papool.tile([P, 336], f32, space="PSUM",
                                                name="acc", tag="acc")
                        mm_done[k] = 0
                    acc = acc_of[k]
                    npair = dn // 2
                    segs = []
                    if npair:
                        nc.vector.tensor_tensor(
                            out=wt[:, soff : soff + npair, 0:TWB],
                            in0=gt[:, off : off + 2 * npair - 1 : 2, 0:TWB],
                            in1=gt[:, off + 1 : off + 2 * npair : 2, 0:TWB],
                            op=OP.add)
                        segs = [wt[:, soff + i, 0:TWB] for i in range(npair)]
                        soff += npair
                    if dn % 2:
                        segs.append(gt[:, off + dn - 1, 0:TWB])
                    for seg in segs:
                        nc.tensor.matmul(acc[:, 0:TWB], ident_t[:], seg,
                                         start=(mm_done[k] == 0),
                                         stop=(mm_done[k] == n_mm[k] - 1),
                                         skip_group_check=True)
                        mm_done[k] += 1
                    if mm_done[k] == n_mm[k]:
                        ready.append((k, acc_of.pop(k)))
                    off += dn

            for ci, ch in enumerate(chunks):
                d_tot = sum(s[2] for s in ch)
                coff = chunk_cols[ci]
                gt = gpool.tile([P, SLOT_CAP, WB], f16, tag="gt2")
                nidx = P * d_tot
                nc.gpsimd.dma_gather(
                    gt[:, 0:d_tot, :], tbl2_d[:],
                    idx2_t[:, coff : coff + 8 * d_tot], nidx, nidx, WB,
                    queue_num=ci % 4,
                )
                wt = wpool.tile([P, 4, TWB], f16, tag="wt2")
                ut = spool.tile([P, SLOT_CAP, 10], f16, tag="ut23")
                off = 0
                for (k, j0, dn) in ch:
                    nc.vector.tensor_tensor(
                        out=ut[:, off : off + dn, :],
                        in0=gt[:, off : off + dn, 320:330],
                        in1=ad23[:, k, :].unsqueeze(1).to_broadcast([P, dn, 10]),
                        op=OP.add,
                    )
                    off += dn
                lt = spool.tile([P, SLOT_CAP, 10], f16, tag="lt23")
                nc.scalar.activation(lt[:, 0:d_tot, :], ut[:, 0:d_tot, :],
                                     AF.Prelu, alpha=NEG)
                nc.scalar.activation(gt[:, 0:d_tot, 320:330],
                                     lt[:, 0:d_tot, :], AF.Exp, bias=ebias[:])
                if pend is not None:
                    weight_and_aggregate_b(*pend)
                    for (k, acc) in ready:
                        epilogue_b(k, acc)
                    ready.clear()
                pend = (ch, gt, wt, ci)
            weight_and_aggregate_b(*pend)
            pend = None
            for (k, acc) in ready:
                epilogue_b(k, acc)
            ready.clear()
            wpool_cm.__exit__(None, None, None)
            gpool_cm.__exit__(None, None, None)
            spool_cm.__exit__(None, None, None)

    nc.compile()
    return nc


# ----------------------------------------------------------------------------
# entry point
# ----------------------------------------------------------------------------
def kernel(x, edge_index, W1, att_src1, att_dst1, b1,
           W2, att_src2, att_dst2, b2,
           W3, att_src3, att_dst3, b3):
    global _compiled
    from concourse.bass_utils import run_bass_kernel_spmd

    x = np.asarray(x, np.float32)
    edge_index = np.asarray(edge_index)

    idx1_all, idx2_all, meta = _prep_graph(edge_index.astype(np.int64))
    chunks, idx_cols = meta["chunks"], meta["idx_cols"]
    D_band = meta["D_band"]

    key = (tuple(tuple(ch) for ch in chunks), idx_cols, D_band)
    if _compiled is None or _compiled[0] != key:
        nc = _build_program(chunks, idx_cols, D_band)
        _compiled = (key, nc)
    nc = _compiled[1]

    # host-side weight augmentation (payload columns in c-major order)
    w1s, w1dst = _w_aug(np.asarray(W1, np.float32), np.asarray(att_src1),
                        np.asarray(att_dst1), HEADS, H1)
    w1big = np.zeros((FIN, WB), np.float32)
    w1big[:, 0:320] = _cmajor(np.asarray(W1, np.float32), HEADS, H1)
    w1big[:, 320:325] = w1s
    w1big[:, 325:330] = w1dst

    w2s, w2dst = _w_aug(np.asarray(W2, np.float32), np.asarray(att_src2),
                        np.asarray(att_dst2), HEADS, H2)
    w3s, w3dst = _w_aug(np.asarray(W3, np.float32), np.asarray(att_src3),
                        np.asarray(att_dst3), HEADS, H2)
    w2big = np.zeros((H1, WB), np.float32)
    w2big[:, 0:160] = _cmajor(np.asarray(W2, np.float32), HEADS, H2)
    w2big[:, 160:320] = _cmajor(np.asarray(W3, np.float32), HEADS, H2)
    w2big[:, 320:325] = w2s
    w2big[:, 325:330] = w3s
    w2big[:, 330:335] = w2dst
    w2big[:, 335:340] = w3dst

    # fp8 sentinel row (table 1): payload 0, fp16 logit halves = -1e4
    sent_row = np.zeros((1, WBB), np.int8)
    sent_row.view(np.float16)[0, 160:170] = -1e4
    # fp16 sentinel row (table 2)
    sent2_row = np.zeros((1, WB), np.float16)
    sent2_row[0, 320:340] = -1e4

    core, band, slot = meta["core"], meta["band"], meta["slot"]
    in_maps = []
    for c in range(NCORE):
        m = core == c
        xT = np.zeros((MLOC, FIN), np.float32)
        xT[band[m] * P + slot[m]] = x[m]
        in_maps.append({
            "xT": np.ascontiguousarray(xT.T).astype(ml_dtypes.bfloat16),
            "w1big": w1big.astype(ml_dtypes.bfloat16),
            "w2big": w2big, "sent": sent_row,
            "sent2": sent2_row,
            "idx1": np.ascontiguousarray(idx1_all[c]),
            "idx2": np.ascontiguousarray(idx2_all[c]),
            "b1r": np.tile(np.asarray(b1, np.float32)[None, :], (P, 1)),
            "b23r": np.tile(np.concatenate([np.asarray(b2, np.float32),
                                            np.asarray(b3, np.float32)])[None, :],
                            (P, 1)),
        })

    global LAST_RESULTS
    res = run_bass_kernel_spmd(nc, in_maps, core_ids=list(range(NCORE)),
                               trace=TRACE, tmpdir=TRACE_DIR)
    LAST_RESULTS = res

    mu = np.empty((N, H2), np.float32)
    lv = np.empty((N, H2), np.float32)
    rows = band * P + slot
    for c in range(NCORE):
        m = core == c
        o = res.results[c]["out"][rows[m]]
        mu[m] = o[:, 0:H2]
        lv[m] = o[:, H2 : 2 * H2]
    return mu, mu.copy(), lv


# revision 49
# speedup vs baseline: 1.0871x; 1.0032x over previous
"""GATModelVAE (2-layer GAT encoder VAE, eval mode) on 8 Trainium2 NeuronCores.

Strategy: destination-node (graph) parallelism. Nodes are packed into
160 windows of 128 dst nodes (degree-sorted, banded so all 8 cores run an
identical program). Per window, incoming edges live in an ELL (slot-major)
layout: slot j of partition n is the j-th in-edge of window-node n; padded
slots point at a sentinel table row whose att-logit columns are -1e4 so
exp() gives exactly 0. Per-edge source features arrive via dma_gather from
an AllGather-replicated table (payload stored c-major i.e. head-minor, and
fp8 for layer 1, converted to f16 on the scalar engine so the DVE alpha-
weighting multiply runs in its fast packed-16-bit 2x mode). The weighting
is one in-place DVE multiply per chunk; the segment sum over edge slots is
one DVE pair-add level followed by PSUM-accumulated identity matmuls (half
the matmul count of slot-at-a-time accumulation), with the exp columns
riding along to yield the softmax denominators. The table build runs in
bf16, and both tables' AllGathers are split into band-blocks scheduled to
hide behind the CC barrier (table 1) and the pass-A window tail (table 2).
Softmax normalization (constant per destination node) is applied after
aggregation in the window epilogue.
"""

import sys

sys.path.insert(0, "/opt/trn_rl_repo")

import numpy as np
import ml_dtypes

N = 20000
E0 = 320000
FIN = 256
H1 = 64
H2 = 32
HEADS = 5
NEG = 0.2

NCORE = 8
P = 128
NWIN = 160            # global windows
NB = NWIN // NCORE    # windows (bands) per core: 20
MLOC = NB * P         # node slots per core: 2560
CONTRIB = MLOC        # per-core AG contribution rows
TROWS = NCORE * MLOC + 8   # + locally-written sentinel row (pad to 8)
SENT = NCORE * MLOC   # sentinel table row
WB = 384              # matmul row width (f32 elems) for the table-build PSUM
WBB = 512             # gathered table-1 row width in BYTES (fp8 payload; %256)
SLOT_CAP = 8          # max ELL slots per gather chunk (1024 idx = 64-desc packet cap)
# exp() is stored in f16 and pair-summed; a constant bias of -ln(16) on the
# exponent scales all numerators AND denominators by 1/16 (cancels in the
# softmax) giving 16x overflow headroom in the f16 partial sums.
EXP_BIAS = -2.772588722239781
# AllGather band-blocks. The first collective can't start before the global
# CC barrier (~50us), so AG1 uses two big blocks; AG2 is front-loaded with a
# small tail so pass B isn't stuck behind a large final AllGather.
BLOCKS1 = ((0, 10), (10, 20))
BLOCKS2 = ((0, 9), (9, 15), (15, 20))

_compiled = None  # (key, nc)
TRACE = False          # set True (e.g. from test.py) to capture an NTFF profile
TRACE_DIR = None       # optional dir for trace artifacts
LAST_RESULTS = None    # BassKernelResults of the most recent run


# ----------------------------------------------------------------------------
# host-side graph preparation
# ----------------------------------------------------------------------------
def _prep_graph(edge_index):
    src = np.concatenate([edge_index[0], np.arange(N, dtype=np.int64)])
    dst = np.concatenate([edge_index[1], np.arange(N, dtype=np.int64)])
    EE = src.shape[0]
    deg = np.bincount(dst, minlength=N)

    order = np.argsort(-deg, kind="stable")      # nodes by degree desc
    pos = np.empty(N, np.int64)
    pos[order] = np.arange(N)
    win = pos // P                               # global window id
    slot = pos % P
    core = win % NCORE
    band = win // NCORE

    # slots per band = max degree in band (shared by all 8 cores)
    D_band = np.zeros(NB, np.int64)
    np.maximum.at(D_band, band, deg)
    D_band = np.maximum(D_band, 1)

    # table rows follow the blocked AG layouts of BLOCKS1 / BLOCKS2
    def blocked_rows(blocks):
        tr = np.empty(N, np.int64)
        for (s, e) in blocks:
            m = (band >= s) & (band < e)
            tr[m] = (NCORE * s * P + core[m] * (e - s) * P
                     + (band[m] - s) * P + slot[m])
        return tr

    trow1 = blocked_rows(BLOCKS1)
    trow2 = blocked_rows(BLOCKS2)

    # per-edge ELL coordinates: (core, band, slot of dst, j = rank among dst's edges)
    eorder = np.argsort(dst, kind="stable")
    ds = dst[eorder]
    run_start = np.r_[0, np.flatnonzero(ds[1:] != ds[:-1]) + 1]
    j_in = np.arange(EE) - np.repeat(run_start, np.diff(np.r_[run_start, EE]))
    es, ed = src[eorder], ds

    ec, eb, eslot = core[ed], band[ed], slot[ed]

    # global chunk layout: chunks of exactly SLOT_CAP slots, crossing band
    # boundaries; each chunk is a list of (band, j0, n_slots) segments
    chunks = []
    cur, cap = [], SLOT_CAP
    for k in range(NB):
        d, j = int(D_band[k]), 0
        while d > 0:
            t = min(cap, d)
            cur.append((k, j, t))
            j += t
            d -= t
            cap -= t
            if cap == 0:
                chunks.append(cur)
                cur, cap = [], SLOT_CAP
    if cur:
        chunks.append(cur)

    # build per-core wrapped int16 index tensors
    idx_cols = sum(8 * sum(s[2] for s in ch) for ch in chunks)

    def build_idx(trow):
        esrc_row = trow[es].astype(np.int32)
        idx_all = np.full((NCORE, 16, idx_cols), SENT, np.int16)
        ell = {}
        for k in range(NB):
            a = np.full((NCORE, int(D_band[k]), P), SENT, np.int32)
            m = eb == k
            a[ec[m], j_in[m], eslot[m]] = esrc_row[m]
            ell[k] = a
        col = 0
        for ch in chunks:
            blk = np.concatenate(
                [ell[k][:, j0 : j0 + dn, :] for (k, j0, dn) in ch], axis=1
            ).reshape(NCORE, -1)
            d_c = sum(s[2] for s in ch)
            wrapped = blk.reshape(NCORE, -1, 16).transpose(0, 2, 1)
            idx_all[:, :, col : col + 8 * d_c] = wrapped.astype(np.int16)
            col += 8 * d_c
        assert col == idx_cols
        return np.tile(idx_all, (1, 8, 1))

    meta = dict(
        chunks=chunks, idx_cols=idx_cols, core=core, band=band, slot=slot,
        D_band=tuple(int(x) for x in D_band),
    )
    return build_idx(trow1), build_idx(trow2), meta


def _w_aug(W, att_s, att_d, heads, hc):
    fin = W.shape[0]
    Wr = W.reshape(fin, heads, hc)
    ws = np.einsum("fhc,hc->fh", Wr, att_s)
    wd = np.einsum("fhc,hc->fh", Wr, att_d)
    return ws.astype(np.float32), wd.astype(np.float32)


def _cmajor(W, heads, hc):
    # [fin, heads*hc] -> columns reordered so col (c*heads + h) = W[:, h*hc + c]
    fin = W.shape[0]
    return np.ascontiguousarray(
        W.reshape(fin, heads, hc).transpose(0, 2, 1).reshape(fin, heads * hc))


# ----------------------------------------------------------------------------
# device program
# ----------------------------------------------------------------------------
def _build_program(chunks, idx_cols, D_band):
    import concourse.bass as bass
    import concourse.bacc as bacc
    import concourse.mybir as mybir
    import concourse.tile as tile
    from concourse import library_config
    from concourse.masks import make_identity

    f32 = mybir.dt.float32
    bf16 = mybir.dt.bfloat16
    f16 = mybir.dt.float16
    f8 = mybir.dt.float8e4
    i8 = mybir.dt.int8
    AF = mybir.ActivationFunctionType
    OP = mybir.AluOpType

    nc = bacc.Bacc("TRN2", target_bir_lowering=False, debug=False,
                   num_devices=NCORE, num_swdge_queues=4)

    xT_d = nc.dram_tensor("xT", [FIN, MLOC], bf16, kind="ExternalInput").ap()
    w1_d = nc.dram_tensor("w1big", [FIN, WB], bf16, kind="ExternalInput").ap()
    w2_d = nc.dram_tensor("w2big", [H1, WB], f32, kind="ExternalInput").ap()
    sent_d = nc.dram_tensor("sent", [1, WBB], i8, kind="ExternalInput").ap()
    sent2_d = nc.dram_tensor("sent2", [1, WB], f16, kind="ExternalInput").ap()
    idx1_d = nc.dram_tensor("idx1", [P, idx_cols], mybir.dt.int16,
                            kind="ExternalInput").ap()
    idx2_d = nc.dram_tensor("idx2", [P, idx_cols], mybir.dt.int16,
                            kind="ExternalInput").ap()
    b1_d = nc.dram_tensor("b1r", [P, H1], f32, kind="ExternalInput").ap()
    b23_d = nc.dram_tensor("b23r", [P, 2 * H2], f32, kind="ExternalInput").ap()

    out_d = nc.dram_tensor("out", [MLOC, 2 * H2], f32,
                           kind="ExternalOutput").ap()

    dum_i = nc.dram_tensor("dumi", [8, 32], i8).ap()
    dum_o = nc.dram_tensor("dumo", [64, 32], i8, addr_space="Shared").ap()
    con1_d = nc.dram_tensor("contrib1", [CONTRIB, WBB], i8).ap()
    con2_d = nc.dram_tensor("contrib2", [CONTRIB, WB], f16).ap()
    tbl1_d = nc.dram_tensor("tbl1", [TROWS, WBB], i8, addr_space="Shared").ap()
    tbl2_d = nc.dram_tensor("tbl2", [TROWS, WB], f16, addr_space="Shared").ap()

    rg = [list(range(NCORE))]

    S_TOT = sum(D_band)                  # total ELL slot columns (352-ish)
    slot0 = [0] * NB                     # first global slot column of band k
    for k in range(1, NB):
        slot0[k] = slot0[k - 1] + D_band[k - 1]

    WA = 328                             # wt col stride pass A (325 used)
    WB2 = 336                            # wt col stride pass B (330 used)
    TWA, TWB = 325, 330                  # tree widths

    with tile.TileContext(nc) as tc:
        with (
            tc.tile_pool(name="const", bufs=1) as cpool,
            tc.tile_pool(name="resid", bufs=1) as rpool,
            tc.tile_pool(name="io", bufs=3) as iopool,
            tc.tile_pool(name="psum", bufs=3, space="PSUM") as pspool,
            tc.tile_pool(name="psumT", bufs=1, space="PSUM") as ptpool,
            tc.tile_pool(name="psumA", bufs=4, space="PSUM") as papool,
        ):
            nc.gpsimd.load_library(library_config.mlp)
            # a tiny dummy AllGather absorbs the one-time CC barrier + DMA
            # ring ramp so the first real AllGather starts without delay
            nc.gpsimd.collective_compute(
                "AllGather", mybir.AluOpType.bypass, replica_groups=rg,
                ins=[dum_i[:]], outs=[dum_o[:]])

            ident = cpool.tile([P, P], f32)
            make_identity(nc, ident[:])
            ident_t = cpool.tile([P, P], f16)
            nc.vector.tensor_copy(ident_t[:], ident[:])
            ebias = cpool.tile([P, 1], f32)
            nc.gpsimd.memset(ebias[:], EXP_BIAS)

            w1_t = cpool.tile([P, 2, WB], bf16)
            nc.sync.dma_start(w1_t[:], w1_d[:].rearrange("(k p) n -> p k n", p=P))
            w2_t = cpool.tile([H1, WB], f32)
            nc.sync.dma_start(w2_t[:], w2_d[:])
            sent_t = cpool.tile([1, WBB], i8)
            nc.sync.dma_start(sent_t[:], sent_d[:])
            sent2_t = cpool.tile([1, WB], f16)
            nc.sync.dma_start(sent2_t[:], sent2_d[:])
            b1_t = cpool.tile([P, H1], f32)
            nc.sync.dma_start(b1_t[:], b1_d[:])
            b23_t = cpool.tile([P, 2 * H2], f32)
            nc.sync.dma_start(b23_t[:], b23_d[:])

            idx1_t = rpool.tile([P, idx_cols], mybir.dt.int16)
            nc.scalar.dma_start(idx1_t[:], idx1_d[:])
            idx2_t = rpool.tile([P, idx_cols], mybir.dt.int16)
            nc.scalar.dma_start(idx2_t[:], idx2_d[:])
            xtpool_cm = tc.tile_pool(name="xt", bufs=1)
            xtpool = xtpool_cm.__enter__()
            xt_all = xtpool.tile([P, 2, MLOC], bf16)
            nc.sync.dma_start(xt_all[:], xT_d[:].rearrange("(k p) n -> p k n", p=P))

            ad1 = rpool.tile([P, NB, 5], f32)
            ad23 = rpool.tile([P, NB, 10], f32)
            h1T = rpool.tile([H1, MLOC], f32)

            # ---------------- pass A: layer-1 table -------------------------
            nc.sync.dma_start(tbl1_d[SENT : SENT + 1, :], sent_t[:])
            nc.sync.dma_start(tbl2_d[SENT : SENT + 1, :], sent2_t[:])
            for m in range(NB):
                ps = pspool.tile([P, WB], f32, space="PSUM", tag="xwps")
                for kk in range(2):
                    nc.tensor.matmul(ps[:], xt_all[:, kk, m * P : (m + 1) * P],
                                     w1_t[:, kk, :],
                                     start=(kk == 0), stop=(kk == 1))
                row_t = iopool.tile([P, WBB], i8, tag="rowt")
                nc.scalar.activation(row_t[:, 0:320].bitcast(f8), ps[:, 0:320],
                                     AF.Copy)
                nc.vector.tensor_copy(row_t[:, 320:330].bitcast(f16),
                                      ps[:, 320:325])
                nc.vector.tensor_copy(ad1[:, m, :], ps[:, 325:330])
                nc.sync.dma_start(con1_d[m * P : (m + 1) * P, :], row_t[:])
                for (s, e) in BLOCKS1:
                    if m == e - 1:
                        nc.gpsimd.collective_compute(
                            "AllGather", mybir.AluOpType.bypass,
                            replica_groups=rg,
                            ins=[con1_d[s * P : e * P, :]],
                            outs=[tbl1_d[NCORE * s * P : NCORE * e * P, :]],
                        )
            # x staging is dead after the table build; release its 20KB
            xtpool_cm.__exit__(None, None, None)

            chunk_cols = []
            chunk_slot0 = []
            col = acc_slots = 0
            for ch in chunks:
                chunk_cols.append(col)
                chunk_slot0.append(acc_slots)
                d_c = sum(s[2] for s in ch)
                col += 8 * d_c
                acc_slots += d_c

            def pair_reduce(wt, soff, off, dn, TW):
                """One DVE pair-add level over wt slots [off, off+dn), cols
                [0,TW), writing pairs to scratch slots starting at soff.
                Returns list of (tile-ish AP) slot sums to feed the PE."""
                outs = []
                npair = dn // 2
                if npair:
                    nc.vector.tensor_tensor(
                        out=wt[:, soff : soff + npair, 0:TW],
                        in0=wt[:, off : off + 2 * npair - 1 : 2, 0:TW],
                        in1=wt[:, off + 1 : off + 2 * npair : 2, 0:TW],
                        op=OP.add)
                    outs = [wt[:, soff + i, 0:TW] for i in range(npair)]
                if dn % 2:
                    outs.append(wt[:, off + dn - 1, 0:TW])
                return outs

            # per band: number of PE accumulation matmuls (pairs + leftovers)
            n_mm = {}
            for ch in chunks:
                for (k, j0, dn) in ch:
                    n_mm[k] = n_mm.get(k, 0) + dn // 2 + dn % 2

            # ---------------- pass A: layer-1 windows -----------------------
            spool_cm = tc.tile_pool(name="small", bufs=8)
            spool = spool_cm.__enter__()
            gpool_cm = tc.tile_pool(name="gatA", bufs=8)
            gpool = gpool_cm.__enter__()
            wpool_cm = tc.tile_pool(name="wtA", bufs=4)
            wpool = wpool_cm.__enter__()

            def epilogue_a(k, acc):
                den = spool.tile([P, 5], f32, tag="den")
                nc.scalar.activation(den[:], acc[:, 320:325], AF.Copy,
                                     scale=float(HEADS), bias=HEADS * 1e-16)
                rec = spool.tile([P, 5], f32, tag="rec")
                nc.vector.reciprocal(rec[:], den[:])
                tmp = spool.tile([P, H1, HEADS], f32, tag="tmp1")
                nc.vector.tensor_tensor(
                    out=tmp[:],
                    in0=acc[:, 0:320].rearrange("p (c h) -> p c h", c=H1),
                    in1=rec[:].unsqueeze(1).to_broadcast([P, H1, HEADS]),
                    op=OP.mult,
                )
                o64 = spool.tile([P, H1], f32, tag="o64")
                nc.vector.tensor_reduce(out=o64[:], in_=tmp[:],
                                        axis=mybir.AxisListType.X, op=OP.add)
                o64b = spool.tile([P, H1], f32, tag="o64b")
                nc.vector.tensor_tensor(out=o64b[:], in0=o64[:], in1=b1_t[:],
                                        op=OP.add)
                nc.scalar.activation(o64[:], o64b[:], AF.Relu)
                pst = ptpool.tile([H1, P], f32, space="PSUM", tag="pst")
                nc.tensor.transpose(pst[:], o64[:], ident[:])
                nc.vector.tensor_copy(h1T[:, k * P : (k + 1) * P], pst[:])
                # layer-2/3 table rows for this band
                ps2 = pspool.tile([P, WB], f32, space="PSUM", tag="xwps")
                nc.tensor.matmul(ps2[:], h1T[:, k * P : (k + 1) * P], w2_t[:],
                                 start=True, stop=True)
                row2_t = iopool.tile([P, WB], f16, tag="rowt2")
                nc.scalar.activation(row2_t[:], ps2[:], AF.Copy)
                nc.vector.tensor_copy(ad23[:, k, :], ps2[:, 330:340])
                nc.sync.dma_start(con2_d[k * P : (k + 1) * P, :], row2_t[:])
                for (s, e) in BLOCKS2:
                    if k == e - 1:
                        nc.gpsimd.collective_compute(
                            "AllGather", mybir.AluOpType.bypass,
                            replica_groups=rg,
                            ins=[con2_d[s * P : e * P, :]],
                            outs=[tbl2_d[NCORE * s * P : NCORE * e * P, :]],
                        )

            # per-chunk: gather -> logit adds -> prelu -> exp -> fp8->f16
            # payload convert (ACT). The alpha-weighting multiply, DVE
            # pair-add level and PE accumulation matmuls run one chunk
            # behind so the DVE never stalls on the ACT round-trip.
            acc_of = {}
            mm_done = {}
            pend = None
            ready = []

            def weight_and_aggregate(ch, gt, wt, ci):
                d_tot = sum(s[2] for s in ch)
                nc.vector.tensor_tensor(
                    out=wt[:, 0:d_tot, 0:320].rearrange(
                        "p d (c h) -> p d c h", c=H1),
                    in0=wt[:, 0:d_tot, 0:320].rearrange(
                        "p d (c h) -> p d c h", c=H1),
                    in1=wt[:, 0:d_tot, 320:325].unsqueeze(2).to_broadcast(
                        [P, d_tot, H1, HEADS]),
                    op=OP.mult,
                )
                off = 0
                soff = 8
                for (k, j0, dn) in ch:
                    if k not in acc_of:
                        acc_of[k] = papool.tile([P, 336], f32, space="PSUM",
                                                name="acc", tag="acc")
                        mm_done[k] = 0
                    acc = acc_of[k]
                    segs = pair_reduce(wt, soff, off, dn, TWA)
                    soff += dn // 2
                    for seg in segs:
                        nc.tensor.matmul(acc[:, 0:TWA], ident_t[:], seg,
                                         start=(mm_done[k] == 0),
                                         stop=(mm_done[k] == n_mm[k] - 1),
                                         skip_group_check=True)
                        mm_done[k] += 1
                    if mm_done[k] == n_mm[k]:
                        ready.append((k, acc_of.pop(k)))
                    off += dn

            for ci, ch in enumerate(chunks):
                d_tot = sum(s[2] for s in ch)
                coff = chunk_cols[ci]
                gt = gpool.tile([P, SLOT_CAP, WBB], i8, tag="gt")
                nidx = P * d_tot
                nc.gpsimd.dma_gather(
                    gt[:, 0:d_tot, :], tbl1_d[:],
                    idx1_t[:, coff : coff + 8 * d_tot], nidx, nidx, WBB,
                    queue_num=ci % 4,
                )
                wt = wpool.tile([P, 12, WA], f16, tag="wt")
                ut = spool.tile([P, SLOT_CAP, 5], f16, tag="ut")
                off = 0
                for (k, j0, dn) in ch:
                    nc.vector.tensor_tensor(
                        out=ut[:, off : off + dn, :],
                        in0=gt[:, off : off + dn, 320:330].bitcast(f16),
                        in1=ad1[:, k, :].unsqueeze(1).to_broadcast([P, dn, 5]),
                        op=OP.add,
                    )
                    off += dn
                lt = spool.tile([P, SLOT_CAP, 5], f16, tag="lt")
                nc.scalar.activation(lt[:, 0:d_tot, :], ut[:, 0:d_tot, :],
                                     AF.Prelu, alpha=NEG)
                nc.scalar.activation(wt[:, 0:d_tot, 320:325],
                                     lt[:, 0:d_tot, :], AF.Exp, bias=ebias[:])
                nc.scalar.activation(wt[:, 0:d_tot, 0:320],
                                     gt[:, 0:d_tot, 0:320].bitcast(f8), AF.Copy)
                if pend is not None:
                    weight_and_aggregate(*pend)
                    for (k, acc) in ready:
                        epilogue_a(k, acc)
                    ready.clear()
                pend = (ch, gt, wt, ci)
            weight_and_aggregate(*pend)
            pend = None
            for (k, acc) in ready:
                epilogue_a(k, acc)
            ready.clear()
            wpool_cm.__exit__(None, None, None)
            gpool_cm.__exit__(None, None, None)

            # ---------------- pass B: layer-2/3 windows ---------------------
            gpool_cm = tc.tile_pool(name="gatB", bufs=8)
            gpool = gpool_cm.__enter__()
            wpool_cm = tc.tile_pool(name="wtB", bufs=3)
            wpool = wpool_cm.__enter__()

            def epilogue_b(k, acc):
                den = spool.tile([P, 10], f32, tag="den23")
                nc.scalar.activation(den[:], acc[:, 320:330], AF.Copy,
                                     scale=float(HEADS), bias=HEADS * 1e-16)
                rec = spool.tile([P, 10], f32, tag="rec23")
                nc.vector.reciprocal(rec[:], den[:])
                tmp = spool.tile([P, 2 * H2, HEADS], f32, tag="tmp2")
                nc.vector.tensor_tensor(
                    out=tmp[:].rearrange("p (l c) h -> p l c h", l=2),
                    in0=acc[:, 0:320].rearrange("p (l c h) -> p l c h",
                                                l=2, c=H2),
                    in1=rec[:].rearrange("p (l h) -> p l h", l=2).unsqueeze(2)
                    .to_broadcast([P, 2, H2, HEADS]),
                    op=OP.mult,
                )
                o64 = spool.tile([P, 2 * H2], f32, tag="o64b2")
                nc.vector.tensor_reduce(out=o64[:], in_=tmp[:],
                                        axis=mybir.AxisListType.X, op=OP.add)
                o64b = spool.tile([P, 2 * H2], f32, tag="o64c2")
                nc.vector.tensor_tensor(out=o64b[:], in0=o64[:], in1=b23_t[:],
                                        op=OP.add)
                nc.sync.dma_start(out_d[k * P : (k + 1) * P, :], o64b[:])

            acc_of = {}
            mm_done = {}
            pend = None
            ready = []

            def weight_and_aggregate_b(ch, gt, wt, ci):
                d_tot = sum(s[2] for s in ch)
                for (li, dsl) in ((0, slice(320, 325)), (1, slice(325, 330))):
                    nc.vector.tensor_tensor(
                        out=gt[:, 0:d_tot, 160 * li : 160 * li + 160].rearrange(
                            "p d (c h) -> p d c h", c=H2),
                        in0=gt[:, 0:d_tot, 160 * li : 160 * li + 160].rearrange(
                            "p d (c h) -> p d c h", c=H2),
                        in1=gt[:, 0:d_tot, dsl].unsqueeze(2).to_broadcast(
                            [P, d_tot, H2, HEADS]),
                        op=OP.mult,
                    )
                off = 0
                soff = 0
                for (k, j0, dn) in ch:
                    if k not in acc_of:
                        acc_of[k] = papool.tile([P, 336], f32, space="PSUM",
                                                name="acc", tag="acc")
                        mm_done[k] = 0
                    acc = acc_of[k]
                    npair = dn // 2
                    segs = []
                    if npair:
                        nc.vector.tensor_tensor(
                            out=wt[:, soff : soff + npair, 0:TWB],
                            in0=gt[:, off : off + 2 * npair - 1 : 2, 0:TWB],
                            in1=gt[:, off + 1 : off + 2 * npair : 2, 0:TWB],
                            op=OP.add)
                        segs = [wt[:, soff + i, 0:TWB] for i in range(npair)]
                        soff += npair
                    if dn % 2:
                        segs.append(gt[:, off + dn - 1, 0:TWB])
                    for seg in segs:
                        nc.tensor.matmul(acc[:, 0:TWB], ident_t[:], seg,
                                         start=(mm_done[k] == 0),
                                         stop=(mm_done[k] == n_mm[k] - 1),
                                         skip_group_check=True)
                        mm_done[k] += 1
                    if mm_done[k] == n_mm[k]:
                        ready.append((k, acc_of.pop(k)))
                    off += dn

            for ci, ch in enumerate(chunks):
                d_tot = sum(s[2] for s in ch)
                coff = chunk_cols[ci]
                gt = gpool.tile([P, SLOT_CAP, WB], f16, tag="gt2")
                nidx = P * d_tot
                nc.gpsimd.dma_gather(
                    gt[:, 0:d_tot, :], tbl2_d[:],
                    idx2_t[:, coff : coff + 8 * d_tot], nidx, nidx, WB,
                    queue_num=ci % 4,
                )
                wt = wpool.tile([P, 4, TWB], f16, tag="wt2")
                ut = spool.tile([P, SLOT_CAP, 10], f16, tag="ut23")
                off = 0
                for (k, j0, dn) in ch:
                    nc.vector.tensor_tensor(
                        out=ut[:, off : off + dn, :],
                        in0=gt[:, off : off + dn, 320:330],
                        in1=ad23[:, k, :].unsqueeze(1).to_broadcast([P, dn, 10]),
                        op=OP.add,
                    )
                    off += dn
                lt = spool.tile([P, SLOT_CAP, 10], f16, tag="lt23")
                nc.scalar.activation(lt[:, 0:d_tot, :], ut[:, 0:d_tot, :],
                                     AF.Prelu, alpha=NEG)
                nc.scalar.activation(gt[:, 0:d_tot, 320:330],
                                     lt[:, 0:d_tot, :], AF.Exp, bias=ebias[:])
                if pend is not None:
                    weight_and_aggregate_b(*pend)
                    for (k, acc) in ready:
                        epilogue_b(k, acc)
                    ready.clear()
                pend = (ch, gt, wt, ci)
            weight_and_aggregate_b(*pend)
            pend = None
            for (k, acc) in ready:
                epilogue_b(k, acc)
            ready.clear()
            wpool_cm.__exit__(None, None, None)
            gpool_cm.__exit__(None, None, None)
            spool_cm.__exit__(None, None, None)

    nc.compile()
    return nc


# ----------------------------------------------------------------------------
# entry point
# ----------------------------------------------------------------------------
def kernel(x, edge_index, W1, att_src1, att_dst1, b1,
           W2, att_src2, att_dst2, b2,
           W3, att_src3, att_dst3, b3):
    global _compiled
    from concourse.bass_utils import run_bass_kernel_spmd

    x = np.asarray(x, np.float32)
    edge_index = np.asarray(edge_index)

    idx1_all, idx2_all, meta = _prep_graph(edge_index.astype(np.int64))
    chunks, idx_cols = meta["chunks"], meta["idx_cols"]
    D_band = meta["D_band"]

    key = (tuple(tuple(ch) for ch in chunks), idx_cols, D_band)
    if _compiled is None or _compiled[0] != key:
        nc = _build_program(chunks, idx_cols, D_band)
        _compiled = (key, nc)
    nc = _compiled[1]

    # host-side weight augmentation (payload columns in c-major order)
    w1s, w1dst = _w_aug(np.asarray(W1, np.float32), np.asarray(att_src1),
                        np.asarray(att_dst1), HEADS, H1)
    w1big = np.zeros((FIN, WB), np.float32)
    w1big[:, 0:320] = _cmajor(np.asarray(W1, np.float32), HEADS, H1)
    w1big[:, 320:325] = w1s
    w1big[:, 325:330] = w1dst

    w2s, w2dst = _w_aug(np.asarray(W2, np.float32), np.asarray(att_src2),
                        np.asarray(att_dst2), HEADS, H2)
    w3s, w3dst = _w_aug(np.asarray(W3, np.float32), np.asarray(att_src3),
                        np.asarray(att_dst3), HEADS, H2)
    w2big = np.zeros((H1, WB), np.float32)
    w2big[:, 0:160] = _cmajor(np.asarray(W2, np.float32), HEADS, H2)
    w2big[:, 160:320] = _cmajor(np.asarray(W3, np.float32), HEADS, H2)
    w2big[:, 320:325] = w2s
    w2big[:, 325:330] = w3s
    w2big[:, 330:335] = w2dst
    w2big[:, 335:340] = w3dst

    # fp8 sentinel row (table 1): payload 0, fp16 logit halves = -1e4
    sent_row = np.zeros((1, WBB), np.int8)
    sent_row.view(np.float16)[0, 160:170] = -1e4
    # fp16 sentinel row (table 2)
    sent2_row = np.zeros((1, WB), np.float16)
    sent2_row[0, 320:340] = -1e4

    core, band, slot = meta["core"], meta["band"], meta["slot"]
    in_maps = []
    for c in range(NCORE):
        m = core == c
        xT = np.zeros((MLOC, FIN), np.float32)
        xT[band[m] * P + slot[m]] = x[m]
        in_maps.append({
            "xT": np.ascontiguousarray(xT.T).astype(ml_dtypes.bfloat16),
            "w1big": w1big.astype(ml_dtypes.bfloat16),
            "w2big": w2big, "sent": sent_row,
            "sent2": sent2_row,
            "idx1": np.ascontiguousarray(idx1_all[c]),
            "idx2": np.ascontiguousarray(idx2_all[c]),
            "b1r": np.tile(np.asarray(b1, np.float32)[None, :], (P, 1)),
            "b23r": np.tile(np.concatenate([np.asarray(b2, np.float32),
                                            np.asarray(b3, np.float32)])[None, :],
                            (P, 1)),
        })

    global LAST_RESULTS
    res = run_bass_kernel_spmd(nc, in_maps, core_ids=list(range(NCORE)),
                               trace=TRACE, tmpdir=TRACE_DIR)
    LAST_RESULTS = res

    mu = np.empty((N, H2), np.float32)
    lv = np.empty((N, H2), np.float32)
    rows = band * P + slot
    for c in range(NCORE):
        m = core == c
        o = res.results[c]["out"][rows[m]]
        mu[m] = o[:, 0:H2]
        lv[m] = o[:, H2 : 2 * H2]
    return mu, mu.copy(), lv


# revision 50
# speedup vs baseline: 1.0956x; 1.0078x over previous
"""GATModelVAE (2-layer GAT encoder VAE, eval mode) on 8 Trainium2 NeuronCores.

Strategy: destination-node (graph) parallelism. Nodes are packed into
160 windows of 128 dst nodes (degree-sorted, banded so all 8 cores run an
identical program). Per window, incoming edges live in an ELL (slot-major)
layout: slot j of partition n is the j-th in-edge of window-node n; padded
slots point at a sentinel table row whose att-logit columns are -1e4 so
exp() gives exactly 0. Per-edge source features arrive via dma_gather from
an AllGather-replicated table (payload stored c-major i.e. head-minor, and
fp8 for layer 1, converted to f16 on the scalar engine so the DVE alpha-
weighting multiply runs in its fast packed-16-bit 2x mode). The weighting
is one in-place DVE multiply per chunk; the segment sum over edge slots is
one DVE pair-add level followed by PSUM-accumulated identity matmuls (half
the matmul count of slot-at-a-time accumulation), with the exp columns
riding along to yield the softmax denominators. The table build runs in
bf16, and both tables' AllGathers are split into band-blocks scheduled to
hide behind the CC barrier (table 1) and the pass-A window tail (table 2).
Softmax normalization (constant per destination node) is applied after
aggregation in the window epilogue.
"""

import sys

sys.path.insert(0, "/opt/trn_rl_repo")

import numpy as np
import ml_dtypes

N = 20000
E0 = 320000
FIN = 256
H1 = 64
H2 = 32
HEADS = 5
NEG = 0.2

NCORE = 8
P = 128
NWIN = 160            # global windows
NB = NWIN // NCORE    # windows (bands) per core: 20
MLOC = NB * P         # node slots per core: 2560
CONTRIB = MLOC        # per-core AG contribution rows
TROWS = NCORE * MLOC + 8   # + locally-written sentinel row (pad to 8)
SENT = NCORE * MLOC   # sentinel table row
WB = 384              # matmul row width (f32 elems) for the table-build PSUM
WBB = 512             # gathered table-1 row width in BYTES (fp8 payload; %256)
SLOT_CAP = 8          # max ELL slots per gather chunk (1024 idx = 64-desc packet cap)
# exp() is stored in f16 and pair-summed; a constant bias of -ln(16) on the
# exponent scales all numerators AND denominators by 1/16 (cancels in the
# softmax) giving 16x overflow headroom in the f16 partial sums.
EXP_BIAS = -2.772588722239781
# AllGather band-blocks. The first collective can't start before the global
# CC barrier (~50us), so AG1 uses two big blocks; AG2 is front-loaded with a
# small tail so pass B isn't stuck behind a large final AllGather.
BLOCKS1 = ((0, 10), (10, 20))
BLOCKS2 = ((0, 9), (9, 15), (15, 20))

_compiled = None  # (key, nc)
TRACE = False          # set True (e.g. from test.py) to capture an NTFF profile
TRACE_DIR = None       # optional dir for trace artifacts
LAST_RESULTS = None    # BassKernelResults of the most recent run


# ----------------------------------------------------------------------------
# host-side graph preparation
# ----------------------------------------------------------------------------
def _prep_graph(edge_index):
    src = np.concatenate([edge_index[0], np.arange(N, dtype=np.int64)])
    dst = np.concatenate([edge_index[1], np.arange(N, dtype=np.int64)])
    EE = src.shape[0]
    deg = np.bincount(dst, minlength=N)

    order = np.argsort(-deg, kind="stable")      # nodes by degree desc
    pos = np.empty(N, np.int64)
    pos[order] = np.arange(N)
    win = pos // P                               # global window id
    slot = pos % P
    core = win % NCORE
    band = win // NCORE

    # slots per band = max degree in band (shared by all 8 cores)
    D_band = np.zeros(NB, np.int64)
    np.maximum.at(D_band, band, deg)
    D_band = np.maximum(D_band, 1)

    # table rows follow the blocked AG layouts of BLOCKS1 / BLOCKS2
    def blocked_rows(blocks):
        tr = np.empty(N, np.int64)
        for (s, e) in blocks:
            m = (band >= s) & (band < e)
            tr[m] = (NCORE * s * P + core[m] * (e - s) * P
                     + (band[m] - s) * P + slot[m])
        return tr

    trow1 = blocked_rows(BLOCKS1)
    trow2 = blocked_rows(BLOCKS2)

    # per-edge ELL coordinates: (core, band, slot of dst, j = rank among dst's edges)
    eorder = np.argsort(dst, kind="stable")
    ds = dst[eorder]
    run_start = np.r_[0, np.flatnonzero(ds[1:] != ds[:-1]) + 1]
    j_in = np.arange(EE) - np.repeat(run_start, np.diff(np.r_[run_start, EE]))
    es, ed = src[eorder], ds

    ec, eb, eslot = core[ed], band[ed], slot[ed]

    # global chunk layout: chunks of exactly SLOT_CAP slots, crossing band
    # boundaries; each chunk is a list of (band, j0, n_slots) segments
    chunks = []
    cur, cap = [], SLOT_CAP
    for k in range(NB):
        d, j = int(D_band[k]), 0
        while d > 0:
            t = min(cap, d)
            cur.append((k, j, t))
            j += t
            d -= t
            cap -= t
            if cap == 0:
                chunks.append(cur)
                cur, cap = [], SLOT_CAP
    if cur:
        chunks.append(cur)

    # build per-core wrapped int16 index tensors
    idx_cols = sum(8 * sum(s[2] for s in ch) for ch in chunks)

    def build_idx(trow):
        esrc_row = trow[es].astype(np.int32)
        idx_all = np.full((NCORE, 16, idx_cols), SENT, np.int16)
        ell = {}
        for k in range(NB):
            a = np.full((NCORE, int(D_band[k]), P), SENT, np.int32)
            m = eb == k
            a[ec[m], j_in[m], eslot[m]] = esrc_row[m]
            ell[k] = a
        col = 0
        for ch in chunks:
            blk = np.concatenate(
                [ell[k][:, j0 : j0 + dn, :] for (k, j0, dn) in ch], axis=1
            ).reshape(NCORE, -1)
            d_c = sum(s[2] for s in ch)
            wrapped = blk.reshape(NCORE, -1, 16).transpose(0, 2, 1)
            idx_all[:, :, col : col + 8 * d_c] = wrapped.astype(np.int16)
            col += 8 * d_c
        assert col == idx_cols
        return np.tile(idx_all, (1, 8, 1))

    meta = dict(
        chunks=chunks, idx_cols=idx_cols, core=core, band=band, slot=slot,
        D_band=tuple(int(x) for x in D_band),
    )
    return build_idx(trow1), build_idx(trow2), meta


def _w_aug(W, att_s, att_d, heads, hc):
    fin = W.shape[0]
    Wr = W.reshape(fin, heads, hc)
    ws = np.einsum("fhc,hc->fh", Wr, att_s)
    wd = np.einsum("fhc,hc->fh", Wr, att_d)
    return ws.astype(np.float32), wd.astype(np.float32)


def _cmajor(W, heads, hc):
    # [fin, heads*hc] -> columns reordered so col (c*heads + h) = W[:, h*hc + c]
    fin = W.shape[0]
    return np.ascontiguousarray(
        W.reshape(fin, heads, hc).transpose(0, 2, 1).reshape(fin, heads * hc))


# ----------------------------------------------------------------------------
# device program
# ----------------------------------------------------------------------------
def _build_program(chunks, idx_cols, D_band):
    import concourse.bass as bass
    import concourse.bacc as bacc
    import concourse.mybir as mybir
    import concourse.tile as tile
    from concourse import library_config
    from concourse.masks import make_identity

    f32 = mybir.dt.float32
    bf16 = mybir.dt.bfloat16
    f16 = mybir.dt.float16
    f8 = mybir.dt.float8e4
    i8 = mybir.dt.int8
    AF = mybir.ActivationFunctionType
    OP = mybir.AluOpType

    nc = bacc.Bacc("TRN2", target_bir_lowering=False, debug=False,
                   num_devices=NCORE, num_swdge_queues=4)

    xT_d = nc.dram_tensor("xT", [FIN, MLOC], bf16, kind="ExternalInput").ap()
    w1_d = nc.dram_tensor("w1big", [FIN, WB], bf16, kind="ExternalInput").ap()
    w2_d = nc.dram_tensor("w2big", [H1, WB], f32, kind="ExternalInput").ap()
    sent_d = nc.dram_tensor("sent", [1, WBB], i8, kind="ExternalInput").ap()
    sent2_d = nc.dram_tensor("sent2", [1, WB], f16, kind="ExternalInput").ap()
    idx1_d = nc.dram_tensor("idx1", [P, idx_cols], mybir.dt.int16,
                            kind="ExternalInput").ap()
    idx2_d = nc.dram_tensor("idx2", [P, idx_cols], mybir.dt.int16,
                            kind="ExternalInput").ap()
    b1_d = nc.dram_tensor("b1r", [P, H1], f32, kind="ExternalInput").ap()
    b23_d = nc.dram_tensor("b23r", [P, 2 * H2], f32, kind="ExternalInput").ap()

    out_d = nc.dram_tensor("out", [MLOC, 2 * H2], f32,
                           kind="ExternalOutput").ap()

    dum_i = nc.dram_tensor("dumi", [8, 32], i8).ap()
    dum_o = nc.dram_tensor("dumo", [64, 32], i8, addr_space="Shared").ap()
    con1_d = nc.dram_tensor("contrib1", [CONTRIB, WBB], i8).ap()
    con2_d = nc.dram_tensor("contrib2", [CONTRIB, WB], f16).ap()
    tbl1_d = nc.dram_tensor("tbl1", [TROWS, WBB], i8, addr_space="Shared").ap()
    tbl2_d = nc.dram_tensor("tbl2", [TROWS, WB], f16, addr_space="Shared").ap()

    rg = [list(range(NCORE))]

    S_TOT = sum(D_band)                  # total ELL slot columns (352-ish)
    slot0 = [0] * NB                     # first global slot column of band k
    for k in range(1, NB):
        slot0[k] = slot0[k - 1] + D_band[k - 1]

    WA = 328                             # wt col stride pass A (325 used)
    WB2 = 336                            # wt col stride pass B (330 used)
    TWA, TWB = 325, 330                  # tree widths

    with tile.TileContext(nc) as tc:
        with (
            tc.tile_pool(name="const", bufs=1) as cpool,
            tc.tile_pool(name="resid", bufs=1) as rpool,
            tc.tile_pool(name="io", bufs=3) as iopool,
            tc.tile_pool(name="psum", bufs=3, space="PSUM") as pspool,
            tc.tile_pool(name="psumT", bufs=1, space="PSUM") as ptpool,
            tc.tile_pool(name="psumA", bufs=4, space="PSUM") as papool,
        ):
            nc.gpsimd.load_library(library_config.mlp)
            # a tiny dummy AllGather absorbs the one-time CC barrier + DMA
            # ring ramp so the first real AllGather starts without delay
            nc.gpsimd.collective_compute(
                "AllGather", mybir.AluOpType.bypass, replica_groups=rg,
                ins=[dum_i[:]], outs=[dum_o[:]])

            ident = cpool.tile([P, P], f32)
            make_identity(nc, ident[:])
            ident_t = cpool.tile([P, P], f16)
            nc.vector.tensor_copy(ident_t[:], ident[:])
            ebias = cpool.tile([P, 1], f32)
            nc.gpsimd.memset(ebias[:], EXP_BIAS)

            w1_t = cpool.tile([P, 2, WB], bf16)
            nc.sync.dma_start(w1_t[:], w1_d[:].rearrange("(k p) n -> p k n", p=P))
            w2_t = cpool.tile([H1, WB], f32)
            nc.sync.dma_start(w2_t[:], w2_d[:])
            sent_t = cpool.tile([1, WBB], i8)
            nc.sync.dma_start(sent_t[:], sent_d[:])
            sent2_t = cpool.tile([1, WB], f16)
            nc.sync.dma_start(sent2_t[:], sent2_d[:])
            b1_t = cpool.tile([P, H1], f32)
            nc.sync.dma_start(b1_t[:], b1_d[:])
            b23_t = cpool.tile([P, 2 * H2], f32)
            nc.sync.dma_start(b23_t[:], b23_d[:])

            idx1_t = rpool.tile([P, idx_cols], mybir.dt.int16)
            nc.scalar.dma_start(idx1_t[:], idx1_d[:])
            idx2_t = rpool.tile([P, idx_cols], mybir.dt.int16)
            nc.scalar.dma_start(idx2_t[:], idx2_d[:])
            xtpool_cm = tc.tile_pool(name="xt", bufs=1)
            xtpool = xtpool_cm.__enter__()
            xt_all = xtpool.tile([P, 2, MLOC], bf16)
            nc.sync.dma_start(xt_all[:], xT_d[:].rearrange("(k p) n -> p k n", p=P))

            ad1 = rpool.tile([P, NB, 5], f32)
            ad23 = rpool.tile([P, NB, 10], f32)
            h1T = rpool.tile([H1, MLOC], f32)

            # ---------------- pass A: layer-1 table -------------------------
            nc.sync.dma_start(tbl1_d[SENT : SENT + 1, :], sent_t[:])
            nc.sync.dma_start(tbl2_d[SENT : SENT + 1, :], sent2_t[:])
            for m in range(NB):
                ps = pspool.tile([P, WB], f32, space="PSUM", tag="xwps")
                for kk in range(2):
                    nc.tensor.matmul(ps[:], xt_all[:, kk, m * P : (m + 1) * P],
                                     w1_t[:, kk, :],
                                     start=(kk == 0), stop=(kk == 1))
                row_t = iopool.tile([P, WBB], i8, tag="rowt")
                nc.scalar.activation(row_t[:, 0:320].bitcast(f8), ps[:, 0:320],
                                     AF.Copy)
                nc.vector.tensor_copy(row_t[:, 320:330].bitcast(f16),
                                      ps[:, 320:325])
                nc.vector.tensor_copy(ad1[:, m, :], ps[:, 325:330])
                nc.sync.dma_start(con1_d[m * P : (m + 1) * P, :], row_t[:])
                for (s, e) in BLOCKS1:
                    if m == e - 1:
                        nc.gpsimd.collective_compute(
                            "AllGather", mybir.AluOpType.bypass,
                            replica_groups=rg,
                            ins=[con1_d[s * P : e * P, :]],
                            outs=[tbl1_d[NCORE * s * P : NCORE * e * P, :]],
                        )
            # x staging is dead after the table build; release its 20KB
            xtpool_cm.__exit__(None, None, None)

            chunk_cols = []
            chunk_slot0 = []
            col = acc_slots = 0
            for ch in chunks:
                chunk_cols.append(col)
                chunk_slot0.append(acc_slots)
                d_c = sum(s[2] for s in ch)
                col += 8 * d_c
                acc_slots += d_c

            def pair_reduce(wt, soff, off, dn, TW):
                """One DVE pair-add level over wt slots [off, off+dn), cols
                [0,TW), writing pairs to scratch slots starting at soff.
                Returns list of (tile-ish AP) slot sums to feed the PE."""
                outs = []
                npair = dn // 2
                if npair:
                    nc.vector.tensor_tensor(
                        out=wt[:, soff : soff + npair, 0:TW],
                        in0=wt[:, off : off + 2 * npair - 1 : 2, 0:TW],
                        in1=wt[:, off + 1 : off + 2 * npair : 2, 0:TW],
                        op=OP.add)
                    outs = [wt[:, soff + i, 0:TW] for i in range(npair)]
                if dn % 2:
                    outs.append(wt[:, off + dn - 1, 0:TW])
                return outs

            # per band: number of PE accumulation matmuls (pairs + leftovers)
            n_mm = {}
            for ch in chunks:
                for (k, j0, dn) in ch:
                    n_mm[k] = n_mm.get(k, 0) + dn // 2 + dn % 2
            direct_b = [ci % 5 < 2 for ci in range(len(chunks))]
            n_mm_b = {}
            for ci, ch in enumerate(chunks):
                for (k, j0, dn) in ch:
                    n_mm_b[k] = n_mm_b.get(k, 0) + (
                        dn if direct_b[ci] else dn // 2 + dn % 2)

            # ---------------- pass A: layer-1 windows -----------------------
            spool_cm = tc.tile_pool(name="small", bufs=8)
            spool = spool_cm.__enter__()
            gpool_cm = tc.tile_pool(name="gatA", bufs=8)
            gpool = gpool_cm.__enter__()
            wpool_cm = tc.tile_pool(name="wtA", bufs=4)
            wpool = wpool_cm.__enter__()

            def epilogue_a(k, acc):
                den = spool.tile([P, 5], f32, tag="den")
                nc.scalar.activation(den[:], acc[:, 320:325], AF.Copy,
                                     scale=float(HEADS), bias=HEADS * 1e-16)
                rec = spool.tile([P, 5], f32, tag="rec")
                nc.vector.reciprocal(rec[:], den[:])
                tmp = spool.tile([P, H1, HEADS], f32, tag="tmp1")
                nc.vector.tensor_tensor(
                    out=tmp[:],
                    in0=acc[:, 0:320].rearrange("p (c h) -> p c h", c=H1),
                    in1=rec[:].unsqueeze(1).to_broadcast([P, H1, HEADS]),
                    op=OP.mult,
                )
                o64 = spool.tile([P, H1], f32, tag="o64")
                nc.vector.tensor_reduce(out=o64[:], in_=tmp[:],
                                        axis=mybir.AxisListType.X, op=OP.add)
                o64b = spool.tile([P, H1], f32, tag="o64b")
                nc.vector.tensor_tensor(out=o64b[:], in0=o64[:], in1=b1_t[:],
                                        op=OP.add)
                nc.scalar.activation(o64[:], o64b[:], AF.Relu)
                pst = ptpool.tile([H1, P], f32, space="PSUM", tag="pst")
                nc.tensor.transpose(pst[:], o64[:], ident[:])
                nc.vector.tensor_copy(h1T[:, k * P : (k + 1) * P], pst[:])
                # layer-2/3 table rows for this band
                ps2 = pspool.tile([P, WB], f32, space="PSUM", tag="xwps")
                nc.tensor.matmul(ps2[:], h1T[:, k * P : (k + 1) * P], w2_t[:],
                                 start=True, stop=True)
                row2_t = iopool.tile([P, WB], f16, tag="rowt2")
                nc.scalar.activation(row2_t[:], ps2[:], AF.Copy)
                nc.vector.tensor_copy(ad23[:, k, :], ps2[:, 330:340])
                nc.sync.dma_start(con2_d[k * P : (k + 1) * P, :], row2_t[:])
                for (s, e) in BLOCKS2:
                    if k == e - 1:
                        nc.gpsimd.collective_compute(
                            "AllGather", mybir.AluOpType.bypass,
                            replica_groups=rg,
                            ins=[con2_d[s * P : e * P, :]],
                            outs=[tbl2_d[NCORE * s * P : NCORE * e * P, :]],
                        )

            # per-chunk: gather -> logit adds -> prelu -> exp -> fp8->f16
            # payload convert (ACT). The alpha-weighting multiply, DVE
            # pair-add level and PE accumulation matmuls run one chunk
            # behind so the DVE never stalls on the ACT round-trip.
            acc_of = {}
            mm_done = {}
            pend = None
            ready = []

            def weight_and_aggregate(ch, gt, wt, ci):
                d_tot = sum(s[2] for s in ch)
                nc.vector.tensor_tensor(
                    out=wt[:, 0:d_tot, 0:320].rearrange(
                        "p d (c h) -> p d c h", c=H1),
                    in0=wt[:, 0:d_tot, 0:320].rearrange(
                        "p d (c h) -> p d c h", c=H1),
                    in1=wt[:, 0:d_tot, 320:325].unsqueeze(2).to_broadcast(
                        [P, d_tot, H1, HEADS]),
                    op=OP.mult,
                )
                off = 0
                soff = 8
                for (k, j0, dn) in ch:
                    if k not in acc_of:
                        acc_of[k] = papool.tile([P, 336], f32, space="PSUM",
                                                name="acc", tag="acc")
                        mm_done[k] = 0
                    acc = acc_of[k]
                    segs = pair_reduce(wt, soff, off, dn, TWA)
                    soff += dn // 2
                    for seg in segs:
                        nc.tensor.matmul(acc[:, 0:TWA], ident_t[:], seg,
                                         start=(mm_done[k] == 0),
                                         stop=(mm_done[k] == n_mm[k] - 1),
                                         skip_group_check=True)
                        mm_done[k] += 1
                    if mm_done[k] == n_mm[k]:
                        ready.append((k, acc_of.pop(k)))
                    off += dn

            for ci, ch in enumerate(chunks):
                d_tot = sum(s[2] for s in ch)
                coff = chunk_cols[ci]
                gt = gpool.tile([P, SLOT_CAP, WBB], i8, tag="gt")
                nidx = P * d_tot
                nc.gpsimd.dma_gather(
                    gt[:, 0:d_tot, :], tbl1_d[:],
                    idx1_t[:, coff : coff + 8 * d_tot], nidx, nidx, WBB,
                    queue_num=ci % 4,
                )
                wt = wpool.tile([P, 12, WA], f16, tag="wt")
                ut = spool.tile([P, SLOT_CAP, 5], f16, tag="ut")
                off = 0
                for (k, j0, dn) in ch:
                    nc.vector.tensor_tensor(
                        out=ut[:, off : off + dn, :],
                        in0=gt[:, off : off + dn, 320:330].bitcast(f16),
                        in1=ad1[:, k, :].unsqueeze(1).to_broadcast([P, dn, 5]),
                        op=OP.add,
                    )
                    off += dn
                lt = spool.tile([P, SLOT_CAP, 5], f16, tag="lt")
                nc.scalar.activation(lt[:, 0:d_tot, :], ut[:, 0:d_tot, :],
                                     AF.Prelu, alpha=NEG)
                nc.scalar.activation(wt[:, 0:d_tot, 320:325],
                                     lt[:, 0:d_tot, :], AF.Exp, bias=ebias[:])
                nc.scalar.activation(wt[:, 0:d_tot, 0:320],
                                     gt[:, 0:d_tot, 0:320].bitcast(f8), AF.Copy)
                if pend is not None:
                    weight_and_aggregate(*pend)
                    for (k, acc) in ready:
                        epilogue_a(k, acc)
                    ready.clear()
                pend = (ch, gt, wt, ci)
            weight_and_aggregate(*pend)
            pend = None
            for (k, acc) in ready:
                epilogue_a(k, acc)
            ready.clear()
            wpool_cm.__exit__(None, None, None)
            gpool_cm.__exit__(None, None, None)

            # ---------------- pass B: layer-2/3 windows ---------------------
            gpool_cm = tc.tile_pool(name="gatB", bufs=8)
            gpool = gpool_cm.__enter__()
            wpool_cm = tc.tile_pool(name="wtB", bufs=3)
            wpool = wpool_cm.__enter__()

            def epilogue_b(k, acc):
                den = spool.tile([P, 10], f32, tag="den23")
                nc.scalar.activation(den[:], acc[:, 320:330], AF.Copy,
                                     scale=float(HEADS), bias=HEADS * 1e-16)
                rec = spool.tile([P, 10], f32, tag="rec23")
                nc.vector.reciprocal(rec[:], den[:])
                tmp = spool.tile([P, 2 * H2, HEADS], f32, tag="tmp2")
                nc.vector.tensor_tensor(
                    out=tmp[:].rearrange("p (l c) h -> p l c h", l=2),
                    in0=acc[:, 0:320].rearrange("p (l c h) -> p l c h",
                                                l=2, c=H2),
                    in1=rec[:].rearrange("p (l h) -> p l h", l=2).unsqueeze(2)
                    .to_broadcast([P, 2, H2, HEADS]),
                    op=OP.mult,
                )
                o64 = spool.tile([P, 2 * H2], f32, tag="o64b2")
                nc.vector.tensor_reduce(out=o64[:], in_=tmp[:],
                                        axis=mybir.AxisListType.X, op=OP.add)
                o64b = spool.tile([P, 2 * H2], f32, tag="o64c2")
                nc.vector.tensor_tensor(out=o64b[:], in0=o64[:], in1=b23_t[:],
                                        op=OP.add)
                nc.sync.dma_start(out_d[k * P : (k + 1) * P, :], o64b[:])

            acc_of = {}
            mm_done = {}
            pend = None
            ready = []

            def weight_and_aggregate_b(ch, gt, wt, ci):
                d_tot = sum(s[2] for s in ch)
                for (li, dsl) in ((0, slice(320, 325)), (1, slice(325, 330))):
                    nc.vector.tensor_tensor(
                        out=gt[:, 0:d_tot, 160 * li : 160 * li + 160].rearrange(
                            "p d (c h) -> p d c h", c=H2),
                        in0=gt[:, 0:d_tot, 160 * li : 160 * li + 160].rearrange(
                            "p d (c h) -> p d c h", c=H2),
                        in1=gt[:, 0:d_tot, dsl].unsqueeze(2).to_broadcast(
                            [P, d_tot, H2, HEADS]),
                        op=OP.mult,
                    )
                off = 0
                soff = 0
                for (k, j0, dn) in ch:
                    if k not in acc_of:
                        acc_of[k] = papool.tile([P, 336], f32, space="PSUM",
                                                name="acc", tag="acc")
                        mm_done[k] = 0
                    acc = acc_of[k]
                    segs = []
                    if direct_b[ci]:
                        segs = [gt[:, off + i, 0:TWB] for i in range(dn)]
                    else:
                        npair = dn // 2
                        if npair:
                            nc.vector.tensor_tensor(
                                out=wt[:, soff : soff + npair, 0:TWB],
                                in0=gt[:, off : off + 2 * npair - 1 : 2, 0:TWB],
                                in1=gt[:, off + 1 : off + 2 * npair : 2, 0:TWB],
                                op=OP.add)
                            segs = [wt[:, soff + i, 0:TWB] for i in range(npair)]
                            soff += npair
                        if dn % 2:
                            segs.append(gt[:, off + dn - 1, 0:TWB])
                    for seg in segs:
                        nc.tensor.matmul(acc[:, 0:TWB], ident_t[:], seg,
                                         start=(mm_done[k] == 0),
                                         stop=(mm_done[k] == n_mm_b[k] - 1),
                                         skip_group_check=True)
                        mm_done[k] += 1
                    if mm_done[k] == n_mm_b[k]:
                        ready.append((k, acc_of.pop(k)))
                    off += dn

            for ci, ch in enumerate(chunks):
                d_tot = sum(s[2] for s in ch)
                coff = chunk_cols[ci]
                gt = gpool.tile([P, SLOT_CAP, WB], f16, tag="gt2")
                nidx = P * d_tot
                nc.gpsimd.dma_gather(
                    gt[:, 0:d_tot, :], tbl2_d[:],
                    idx2_t[:, coff : coff + 8 * d_tot], nidx, nidx, WB,
                    queue_num=ci % 4,
                )
                wt = wpool.tile([P, 4, TWB], f16, tag="wt2")
                ut = spool.tile([P, SLOT_CAP, 10], f16, tag="ut23")
                off = 0
                for (k, j0, dn) in ch:
                    nc.vector.tensor_tensor(
                        out=ut[:, off : off + dn, :],
                        in0=gt[:, off : off + dn, 320:330],
                        in1=ad23[:, k, :].unsqueeze(1).to_broadcast([P, dn, 10]),
                        op=OP.add,
                    )
                    off += dn
                lt = spool.tile([P, SLOT_CAP, 10], f16, tag="lt23")
                nc.scalar.activation(lt[:, 0:d_tot, :], ut[:, 0:d_tot, :],
                                     AF.Prelu, alpha=NEG)
                nc.scalar.activation(gt[:, 0:d_tot, 320:330],
                                     lt[:, 0:d_tot, :], AF.Exp, bias=ebias[:])
                if pend is not None:
                    weight_and_aggregate_b(*pend)
                    for (k, acc) in ready:
                        epilogue_b(k, acc)
                    ready.clear()
                pend = (ch, gt, wt, ci)
            weight_and_aggregate_b(*pend)
            pend = None
            for (k, acc) in ready:
                epilogue_b(k, acc)
            ready.clear()
            wpool_cm.__exit__(None, None, None)
            gpool_cm.__exit__(None, None, None)
            spool_cm.__exit__(None, None, None)

    nc.compile()
    return nc


# ----------------------------------------------------------------------------
# entry point
# ----------------------------------------------------------------------------
def kernel(x, edge_index, W1, att_src1, att_dst1, b1,
           W2, att_src2, att_dst2, b2,
           W3, att_src3, att_dst3, b3):
    global _compiled
    from concourse.bass_utils import run_bass_kernel_spmd

    x = np.asarray(x, np.float32)
    edge_index = np.asarray(edge_index)

    idx1_all, idx2_all, meta = _prep_graph(edge_index.astype(np.int64))
    chunks, idx_cols = meta["chunks"], meta["idx_cols"]
    D_band = meta["D_band"]

    key = (tuple(tuple(ch) for ch in chunks), idx_cols, D_band)
    if _compiled is None or _compiled[0] != key:
        nc = _build_program(chunks, idx_cols, D_band)
        _compiled = (key, nc)
    nc = _compiled[1]

    # host-side weight augmentation (payload columns in c-major order)
    w1s, w1dst = _w_aug(np.asarray(W1, np.float32), np.asarray(att_src1),
                        np.asarray(att_dst1), HEADS, H1)
    w1big = np.zeros((FIN, WB), np.float32)
    w1big[:, 0:320] = _cmajor(np.asarray(W1, np.float32), HEADS, H1)
    w1big[:, 320:325] = w1s
    w1big[:, 325:330] = w1dst

    w2s, w2dst = _w_aug(np.asarray(W2, np.float32), np.asarray(att_src2),
                        np.asarray(att_dst2), HEADS, H2)
    w3s, w3dst = _w_aug(np.asarray(W3, np.float32), np.asarray(att_src3),
                        np.asarray(att_dst3), HEADS, H2)
    w2big = np.zeros((H1, WB), np.float32)
    w2big[:, 0:160] = _cmajor(np.asarray(W2, np.float32), HEADS, H2)
    w2big[:, 160:320] = _cmajor(np.asarray(W3, np.float32), HEADS, H2)
    w2big[:, 320:325] = w2s
    w2big[:, 325:330] = w3s
    w2big[:, 330:335] = w2dst
    w2big[:, 335:340] = w3dst

    # fp8 sentinel row (table 1): payload 0, fp16 logit halves = -1e4
    sent_row = np.zeros((1, WBB), np.int8)
    sent_row.view(np.float16)[0, 160:170] = -1e4
    # fp16 sentinel row (table 2)
    sent2_row = np.zeros((1, WB), np.float16)
    sent2_row[0, 320:340] = -1e4

    core, band, slot = meta["core"], meta["band"], meta["slot"]
    in_maps = []
    for c in range(NCORE):
        m = core == c
        xT = np.zeros((MLOC, FIN), np.float32)
        xT[band[m] * P + slot[m]] = x[m]
        in_maps.append({
            "xT": np.ascontiguousarray(xT.T).astype(ml_dtypes.bfloat16),
            "w1big": w1big.astype(ml_dtypes.bfloat16),
            "w2big": w2big, "sent": sent_row,
            "sent2": sent2_row,
            "idx1": np.ascontiguousarray(idx1_all[c]),
            "idx2": np.ascontiguousarray(idx2_all[c]),
            "b1r": np.tile(np.asarray(b1, np.float32)[None, :], (P, 1)),
            "b23r": np.tile(np.concatenate([np.asarray(b2, np.float32),
                                            np.asarray(b3, np.float32)])[None, :],
                            (P, 1)),
        })

    global LAST_RESULTS
    res = run_bass_kernel_spmd(nc, in_maps, core_ids=list(range(NCORE)),
                               trace=TRACE, tmpdir=TRACE_DIR)
    LAST_RESULTS = res

    mu = np.empty((N, H2), np.float32)
    lv = np.empty((N, H2), np.float32)
    rows = band * P + slot
    for c in range(NCORE):
        m = core == c
        o = res.results[c]["out"][rows[m]]
        mu[m] = o[:, 0:H2]
        lv[m] = o[:, H2 : 2 * H2]
    return mu, mu.copy(), lv


# revision 51
# speedup vs baseline: 1.1034x; 1.0071x over previous
"""GATModelVAE (2-layer GAT encoder VAE, eval mode) on 8 Trainium2 NeuronCores.

Strategy: destination-node (graph) parallelism. Nodes are packed into
160 windows of 128 dst nodes (degree-sorted, banded so all 8 cores run an
identical program). Per window, incoming edges live in an ELL (slot-major)
layout: slot j of partition n is the j-th in-edge of window-node n; padded
slots point at a sentinel table row whose att-logit columns are -1e4 so
exp() gives exactly 0. Per-edge source features arrive via dma_gather from
an AllGather-replicated table (payload stored c-major i.e. head-minor, and
fp8 for layer 1, converted to f16 on the scalar engine so the DVE alpha-
weighting multiply runs in its fast packed-16-bit 2x mode). The weighting
is one in-place DVE multiply per chunk; the segment sum over edge slots is
one DVE pair-add level followed by PSUM-accumulated identity matmuls (half
the matmul count of slot-at-a-time accumulation), with the exp columns
riding along to yield the softmax denominators. The table build runs in
bf16, and both tables' AllGathers are split into band-blocks scheduled to
hide behind the CC barrier (table 1) and the pass-A window tail (table 2).
Softmax normalization (constant per destination node) is applied after
aggregation in the window epilogue.
"""

import sys

sys.path.insert(0, "/opt/trn_rl_repo")

import numpy as np
import ml_dtypes

N = 20000
E0 = 320000
FIN = 256
H1 = 64
H2 = 32
HEADS = 5
NEG = 0.2

NCORE = 8
P = 128
NWIN = 160            # global windows
NB = NWIN // NCORE    # windows (bands) per core: 20
MLOC = NB * P         # node slots per core: 2560
CONTRIB = MLOC        # per-core AG contribution rows
TROWS = NCORE * MLOC + 8   # + locally-written sentinel row (pad to 8)
SENT = NCORE * MLOC   # sentinel table row
WB = 384              # matmul row width (f32 elems) for the table-build PSUM
WBB = 512             # gathered table-1 row width in BYTES (fp8 payload; %256)
SLOT_CAP = 8          # max ELL slots per gather chunk (1024 idx = 64-desc packet cap)
# exp() is stored in f16 and pair-summed; a constant bias of -ln(16) on the
# exponent scales all numerators AND denominators by 1/16 (cancels in the
# softmax) giving 16x overflow headroom in the f16 partial sums.
EXP_BIAS = -2.772588722239781
# AllGather band-blocks. The first collective can't start before the global
# CC barrier (~50us), so AG1 uses two big blocks; AG2 is front-loaded with a
# small tail so pass B isn't stuck behind a large final AllGather.
BLOCKS1 = ((0, 10), (10, 20))
BLOCKS2 = ((0, 9), (9, 15), (15, 20))

_compiled = None  # (key, nc)
TRACE = False          # set True (e.g. from test.py) to capture an NTFF profile
TRACE_DIR = None       # optional dir for trace artifacts
LAST_RESULTS = None    # BassKernelResults of the most recent run


# ----------------------------------------------------------------------------
# host-side graph preparation
# ----------------------------------------------------------------------------
def _prep_graph(edge_index):
    src = np.concatenate([edge_index[0], np.arange(N, dtype=np.int64)])
    dst = np.concatenate([edge_index[1], np.arange(N, dtype=np.int64)])
    EE = src.shape[0]
    deg = np.bincount(dst, minlength=N)

    order = np.argsort(-deg, kind="stable")      # nodes by degree desc
    pos = np.empty(N, np.int64)
    pos[order] = np.arange(N)
    win = pos // P                               # global window id
    slot = pos % P
    core = win % NCORE
    band = win // NCORE

    # slots per band = max degree in band (shared by all 8 cores)
    D_band = np.zeros(NB, np.int64)
    np.maximum.at(D_band, band, deg)
    D_band = np.maximum(D_band, 1)

    # table rows follow the blocked AG layouts of BLOCKS1 / BLOCKS2
    def blocked_rows(blocks):
        tr = np.empty(N, np.int64)
        for (s, e) in blocks:
            m = (band >= s) & (band < e)
            tr[m] = (NCORE * s * P + core[m] * (e - s) * P
                     + (band[m] - s) * P + slot[m])
        return tr

    trow1 = blocked_rows(BLOCKS1)
    trow2 = blocked_rows(BLOCKS2)

    # per-edge ELL coordinates: (core, band, slot of dst, j = rank among dst's edges)
    eorder = np.argsort(dst, kind="stable")
    ds = dst[eorder]
    run_start = np.r_[0, np.flatnonzero(ds[1:] != ds[:-1]) + 1]
    j_in = np.arange(EE) - np.repeat(run_start, np.diff(np.r_[run_start, EE]))
    es, ed = src[eorder], ds

    ec, eb, eslot = core[ed], band[ed], slot[ed]

    # global chunk layout: chunks of exactly SLOT_CAP slots, crossing band
    # boundaries; each chunk is a list of (band, j0, n_slots) segments
    chunks = []
    cur, cap = [], SLOT_CAP
    for k in range(NB):
        d, j = int(D_band[k]), 0
        while d > 0:
            t = min(cap, d)
            cur.append((k, j, t))
            j += t
            d -= t
            cap -= t
            if cap == 0:
                chunks.append(cur)
                cur, cap = [], SLOT_CAP
    if cur:
        chunks.append(cur)

    # build per-core wrapped int16 index tensors
    idx_cols = sum(8 * sum(s[2] for s in ch) for ch in chunks)

    def build_idx(trow):
        esrc_row = trow[es].astype(np.int32)
        idx_all = np.full((NCORE, 16, idx_cols), SENT, np.int16)
        ell = {}
        for k in range(NB):
            a = np.full((NCORE, int(D_band[k]), P), SENT, np.int32)
            m = eb == k
            a[ec[m], j_in[m], eslot[m]] = esrc_row[m]
            ell[k] = a
        col = 0
        for ch in chunks:
            blk = np.concatenate(
                [ell[k][:, j0 : j0 + dn, :] for (k, j0, dn) in ch], axis=1
            ).reshape(NCORE, -1)
            d_c = sum(s[2] for s in ch)
            wrapped = blk.reshape(NCORE, -1, 16).transpose(0, 2, 1)
            idx_all[:, :, col : col + 8 * d_c] = wrapped.astype(np.int16)
            col += 8 * d_c
        assert col == idx_cols
        return np.tile(idx_all, (1, 8, 1))

    meta = dict(
        chunks=chunks, idx_cols=idx_cols, core=core, band=band, slot=slot,
        D_band=tuple(int(x) for x in D_band),
    )
    return build_idx(trow1), build_idx(trow2), meta


def _w_aug(W, att_s, att_d, heads, hc):
    fin = W.shape[0]
    Wr = W.reshape(fin, heads, hc)
    ws = np.einsum("fhc,hc->fh", Wr, att_s)
    wd = np.einsum("fhc,hc->fh", Wr, att_d)
    return ws.astype(np.float32), wd.astype(np.float32)


def _cmajor(W, heads, hc):
    # [fin, heads*hc] -> columns reordered so col (c*heads + h) = W[:, h*hc + c]
    fin = W.shape[0]
    return np.ascontiguousarray(
        W.reshape(fin, heads, hc).transpose(0, 2, 1).reshape(fin, heads * hc))


# ----------------------------------------------------------------------------
# device program
# ----------------------------------------------------------------------------
def _build_program(chunks, idx_cols, D_band):
    import concourse.bass as bass
    import concourse.bacc as bacc
    import concourse.mybir as mybir
    import concourse.tile as tile
    from concourse import library_config
    from concourse.masks import make_identity

    f32 = mybir.dt.float32
    bf16 = mybir.dt.bfloat16
    f16 = mybir.dt.float16
    f8 = mybir.dt.float8e4
    i8 = mybir.dt.int8
    AF = mybir.ActivationFunctionType
    OP = mybir.AluOpType

    nc = bacc.Bacc("TRN2", target_bir_lowering=False, debug=False,
                   num_devices=NCORE, num_swdge_queues=4)

    xT_d = nc.dram_tensor("xT", [FIN, MLOC], bf16, kind="ExternalInput").ap()
    w1_d = nc.dram_tensor("w1big", [FIN, WB], bf16, kind="ExternalInput").ap()
    w2_d = nc.dram_tensor("w2big", [H1, WB], f32, kind="ExternalInput").ap()
    sent_d = nc.dram_tensor("sent", [1, WBB], i8, kind="ExternalInput").ap()
    sent2_d = nc.dram_tensor("sent2", [1, WB], f16, kind="ExternalInput").ap()
    idx1_d = nc.dram_tensor("idx1", [P, idx_cols], mybir.dt.int16,
                            kind="ExternalInput").ap()
    idx2_d = nc.dram_tensor("idx2", [P, idx_cols], mybir.dt.int16,
                            kind="ExternalInput").ap()
    b1_d = nc.dram_tensor("b1r", [P, H1], f32, kind="ExternalInput").ap()
    b23_d = nc.dram_tensor("b23r", [P, 2 * H2], f32, kind="ExternalInput").ap()

    out_d = nc.dram_tensor("out", [MLOC, 2 * H2], f32,
                           kind="ExternalOutput").ap()

    dum_i = nc.dram_tensor("dumi", [8, 32], i8).ap()
    dum_o = nc.dram_tensor("dumo", [64, 32], i8, addr_space="Shared").ap()
    con1_d = nc.dram_tensor("contrib1", [CONTRIB, WBB], i8).ap()
    con2_d = nc.dram_tensor("contrib2", [CONTRIB, WB], f16).ap()
    tbl1_d = nc.dram_tensor("tbl1", [TROWS, WBB], i8, addr_space="Shared").ap()
    tbl2_d = nc.dram_tensor("tbl2", [TROWS, WB], f16, addr_space="Shared").ap()

    rg = [list(range(NCORE))]

    S_TOT = sum(D_band)                  # total ELL slot columns (352-ish)
    slot0 = [0] * NB                     # first global slot column of band k
    for k in range(1, NB):
        slot0[k] = slot0[k - 1] + D_band[k - 1]

    WA = 328                             # wt col stride pass A (325 used)
    WB2 = 336                            # wt col stride pass B (330 used)
    TWA, TWB = 325, 330                  # tree widths

    with tile.TileContext(nc) as tc:
        with (
            tc.tile_pool(name="const", bufs=1) as cpool,
            tc.tile_pool(name="resid", bufs=1) as rpool,
            tc.tile_pool(name="io", bufs=3) as iopool,
            tc.tile_pool(name="psum", bufs=3, space="PSUM") as pspool,
            tc.tile_pool(name="psumT", bufs=1, space="PSUM") as ptpool,
            tc.tile_pool(name="psumA", bufs=4, space="PSUM") as papool,
        ):
            nc.gpsimd.load_library(library_config.mlp)
            # a tiny dummy AllGather absorbs the one-time CC barrier + DMA
            # ring ramp so the first real AllGather starts without delay
            nc.gpsimd.collective_compute(
                "AllGather", mybir.AluOpType.bypass, replica_groups=rg,
                ins=[dum_i[:]], outs=[dum_o[:]])

            ident = cpool.tile([P, P], f32)
            make_identity(nc, ident[:])
            ident_t = cpool.tile([P, P], f16)
            nc.vector.tensor_copy(ident_t[:], ident[:])
            ebias = cpool.tile([P, 1], f32)
            nc.gpsimd.memset(ebias[:], EXP_BIAS)

            w1_t = cpool.tile([P, 2, WB], bf16)
            nc.sync.dma_start(w1_t[:], w1_d[:].rearrange("(k p) n -> p k n", p=P))
            w2_t = cpool.tile([H1, WB], f32)
            nc.sync.dma_start(w2_t[:], w2_d[:])
            sent_t = cpool.tile([1, WBB], i8)
            nc.sync.dma_start(sent_t[:], sent_d[:])
            sent2_t = cpool.tile([1, WB], f16)
            nc.sync.dma_start(sent2_t[:], sent2_d[:])
            b1_t = cpool.tile([P, H1], f32)
            nc.sync.dma_start(b1_t[:], b1_d[:])
            b23_t = cpool.tile([P, 2 * H2], f32)
            nc.sync.dma_start(b23_t[:], b23_d[:])

            idx1_t = rpool.tile([P, idx_cols], mybir.dt.int16)
            nc.scalar.dma_start(idx1_t[:], idx1_d[:])
            idx2_t = rpool.tile([P, idx_cols], mybir.dt.int16)
            nc.scalar.dma_start(idx2_t[:], idx2_d[:])
            xtpool_cm = tc.tile_pool(name="xt", bufs=1)
            xtpool = xtpool_cm.__enter__()
            xt_all = xtpool.tile([P, 2, MLOC], bf16)
            nc.sync.dma_start(xt_all[:], xT_d[:].rearrange("(k p) n -> p k n", p=P))

            ad1 = rpool.tile([P, NB, 5], f32)
            ad23 = rpool.tile([P, NB, 10], f32)
            h1T = rpool.tile([H1, MLOC], f32)

            # ---------------- pass A: layer-1 table -------------------------
            nc.sync.dma_start(tbl1_d[SENT : SENT + 1, :], sent_t[:])
            nc.sync.dma_start(tbl2_d[SENT : SENT + 1, :], sent2_t[:])
            for m in range(NB):
                ps = pspool.tile([P, WB], f32, space="PSUM", tag="xwps")
                for kk in range(2):
                    nc.tensor.matmul(ps[:], xt_all[:, kk, m * P : (m + 1) * P],
                                     w1_t[:, kk, :],
                                     start=(kk == 0), stop=(kk == 1))
                row_t = iopool.tile([P, WBB], i8, tag="rowt")
                nc.scalar.activation(row_t[:, 0:320].bitcast(f8), ps[:, 0:320],
                                     AF.Copy)
                nc.vector.tensor_copy(row_t[:, 320:330].bitcast(f16),
                                      ps[:, 320:325])
                nc.vector.tensor_copy(ad1[:, m, :], ps[:, 325:330])
                nc.sync.dma_start(con1_d[m * P : (m + 1) * P, :], row_t[:])
                for (s, e) in BLOCKS1:
                    if m == e - 1:
                        nc.gpsimd.collective_compute(
                            "AllGather", mybir.AluOpType.bypass,
                            replica_groups=rg,
                            ins=[con1_d[s * P : e * P, :]],
                            outs=[tbl1_d[NCORE * s * P : NCORE * e * P, :]],
                        )
            # x staging is dead after the table build; release its 20KB
            xtpool_cm.__exit__(None, None, None)

            chunk_cols = []
            chunk_slot0 = []
            col = acc_slots = 0
            for ch in chunks:
                chunk_cols.append(col)
                chunk_slot0.append(acc_slots)
                d_c = sum(s[2] for s in ch)
                col += 8 * d_c
                acc_slots += d_c

            def pair_reduce(wt, soff, off, dn, TW):
                """One DVE pair-add level over wt slots [off, off+dn), cols
                [0,TW), writing pairs to scratch slots starting at soff.
                Returns list of (tile-ish AP) slot sums to feed the PE."""
                outs = []
                npair = dn // 2
                if npair:
                    nc.vector.tensor_tensor(
                        out=wt[:, soff : soff + npair, 0:TW],
                        in0=wt[:, off : off + 2 * npair - 1 : 2, 0:TW],
                        in1=wt[:, off + 1 : off + 2 * npair : 2, 0:TW],
                        op=OP.add)
                    outs = [wt[:, soff + i, 0:TW] for i in range(npair)]
                if dn % 2:
                    outs.append(wt[:, off + dn - 1, 0:TW])
                return outs

            # per band: number of PE accumulation matmuls (pairs + leftovers)
            n_mm = {}
            for ch in chunks:
                for (k, j0, dn) in ch:
                    n_mm[k] = n_mm.get(k, 0) + dn // 2 + dn % 2
            direct_b = [ci % 5 < 3 for ci in range(len(chunks))]
            n_mm_b = {}
            for ci, ch in enumerate(chunks):
                for (k, j0, dn) in ch:
                    n_mm_b[k] = n_mm_b.get(k, 0) + (
                        dn if direct_b[ci] else dn // 2 + dn % 2)

            # ---------------- pass A: layer-1 windows -----------------------
            spool_cm = tc.tile_pool(name="small", bufs=8)
            spool = spool_cm.__enter__()
            gpool_cm = tc.tile_pool(name="gatA", bufs=8)
            gpool = gpool_cm.__enter__()
            wpool_cm = tc.tile_pool(name="wtA", bufs=4)
            wpool = wpool_cm.__enter__()

            def epilogue_a(k, acc):
                den = spool.tile([P, 5], f32, tag="den")
                nc.scalar.activation(den[:], acc[:, 320:325], AF.Copy,
                                     scale=float(HEADS), bias=HEADS * 1e-16)
                rec = spool.tile([P, 5], f32, tag="rec")
                nc.vector.reciprocal(rec[:], den[:])
                tmp = spool.tile([P, H1, HEADS], f32, tag="tmp1")
                nc.vector.tensor_tensor(
                    out=tmp[:],
                    in0=acc[:, 0:320].rearrange("p (c h) -> p c h", c=H1),
                    in1=rec[:].unsqueeze(1).to_broadcast([P, H1, HEADS]),
                    op=OP.mult,
                )
                o64 = spool.tile([P, H1], f32, tag="o64")
                nc.vector.tensor_reduce(out=o64[:], in_=tmp[:],
                                        axis=mybir.AxisListType.X, op=OP.add)
                o64b = spool.tile([P, H1], f32, tag="o64b")
                nc.vector.tensor_tensor(out=o64b[:], in0=o64[:], in1=b1_t[:],
                                        op=OP.add)
                nc.scalar.activation(o64[:], o64b[:], AF.Relu)
                pst = ptpool.tile([H1, P], f32, space="PSUM", tag="pst")
                nc.tensor.transpose(pst[:], o64[:], ident[:])
                nc.vector.tensor_copy(h1T[:, k * P : (k + 1) * P], pst[:])
                # layer-2/3 table rows for this band
                ps2 = pspool.tile([P, WB], f32, space="PSUM", tag="xwps")
                nc.tensor.matmul(ps2[:], h1T[:, k * P : (k + 1) * P], w2_t[:],
                                 start=True, stop=True)
                row2_t = iopool.tile([P, WB], f16, tag="rowt2")
                nc.scalar.activation(row2_t[:], ps2[:], AF.Copy)
                nc.vector.tensor_copy(ad23[:, k, :], ps2[:, 330:340])
                nc.sync.dma_start(con2_d[k * P : (k + 1) * P, :], row2_t[:])
                for (s, e) in BLOCKS2:
                    if k == e - 1:
                        nc.gpsimd.collective_compute(
                            "AllGather", mybir.AluOpType.bypass,
                            replica_groups=rg,
                            ins=[con2_d[s * P : e * P, :]],
                            outs=[tbl2_d[NCORE * s * P : NCORE * e * P, :]],
                        )

            # per-chunk: gather -> logit adds -> prelu -> exp -> fp8->f16
            # payload convert (ACT). The alpha-weighting multiply, DVE
            # pair-add level and PE accumulation matmuls run one chunk
            # behind so the DVE never stalls on the ACT round-trip.
            acc_of = {}
            mm_done = {}
            pend = None
            ready = []

            def weight_and_aggregate(ch, gt, wt, ci):
                d_tot = sum(s[2] for s in ch)
                nc.vector.tensor_tensor(
                    out=wt[:, 0:d_tot, 0:320].rearrange(
                        "p d (c h) -> p d c h", c=H1),
                    in0=wt[:, 0:d_tot, 0:320].rearrange(
                        "p d (c h) -> p d c h", c=H1),
                    in1=wt[:, 0:d_tot, 320:325].unsqueeze(2).to_broadcast(
                        [P, d_tot, H1, HEADS]),
                    op=OP.mult,
                )
                off = 0
                soff = 8
                for (k, j0, dn) in ch:
                    if k not in acc_of:
                        acc_of[k] = papool.tile([P, 336], f32, space="PSUM",
                                                name="acc", tag="acc")
                        mm_done[k] = 0
                    acc = acc_of[k]
                    segs = pair_reduce(wt, soff, off, dn, TWA)
                    soff += dn // 2
                    for seg in segs:
                        nc.tensor.matmul(acc[:, 0:TWA], ident_t[:], seg,
                                         start=(mm_done[k] == 0),
                                         stop=(mm_done[k] == n_mm[k] - 1),
                                         skip_group_check=True)
                        mm_done[k] += 1
                    if mm_done[k] == n_mm[k]:
                        ready.append((k, acc_of.pop(k)))
                    off += dn

            for ci, ch in enumerate(chunks):
                d_tot = sum(s[2] for s in ch)
                coff = chunk_cols[ci]
                gt = gpool.tile([P, SLOT_CAP, WBB], i8, tag="gt")
                nidx = P * d_tot
                nc.gpsimd.dma_gather(
                    gt[:, 0:d_tot, :], tbl1_d[:],
                    idx1_t[:, coff : coff + 8 * d_tot], nidx, nidx, WBB,
                    queue_num=ci % 4,
                )
                wt = wpool.tile([P, 12, WA], f16, tag="wt")
                ut = spool.tile([P, SLOT_CAP, 5], f16, tag="ut")
                off = 0
                for (k, j0, dn) in ch:
                    nc.vector.tensor_tensor(
                        out=ut[:, off : off + dn, :],
                        in0=gt[:, off : off + dn, 320:330].bitcast(f16),
                        in1=ad1[:, k, :].unsqueeze(1).to_broadcast([P, dn, 5]),
                        op=OP.add,
                    )
                    off += dn
                lt = spool.tile([P, SLOT_CAP, 5], f16, tag="lt")
                nc.scalar.activation(lt[:, 0:d_tot, :], ut[:, 0:d_tot, :],
                                     AF.Prelu, alpha=NEG)
                nc.scalar.activation(wt[:, 0:d_tot, 320:325],
                                     lt[:, 0:d_tot, :], AF.Exp, bias=ebias[:])
                nc.scalar.activation(wt[:, 0:d_tot, 0:320],
                                     gt[:, 0:d_tot, 0:320].bitcast(f8), AF.Copy)
                if pend is not None:
                    weight_and_aggregate(*pend)
                    for (k, acc) in ready:
                        epilogue_a(k, acc)
                    ready.clear()
                pend = (ch, gt, wt, ci)
            weight_and_aggregate(*pend)
            pend = None
            for (k, acc) in ready:
                epilogue_a(k, acc)
            ready.clear()
            wpool_cm.__exit__(None, None, None)
            gpool_cm.__exit__(None, None, None)

            # ---------------- pass B: layer-2/3 windows ---------------------
            gpool_cm = tc.tile_pool(name="gatB", bufs=8)
            gpool = gpool_cm.__enter__()
            wpool_cm = tc.tile_pool(name="wtB", bufs=3)
            wpool = wpool_cm.__enter__()

            def epilogue_b(k, acc):
                den = spool.tile([P, 10], f32, tag="den23")
                nc.scalar.activation(den[:], acc[:, 320:330], AF.Copy,
                                     scale=float(HEADS), bias=HEADS * 1e-16)
                rec = spool.tile([P, 10], f32, tag="rec23")
                nc.vector.reciprocal(rec[:], den[:])
                tmp = spool.tile([P, 2 * H2, HEADS], f32, tag="tmp2")
                nc.vector.tensor_tensor(
                    out=tmp[:].rearrange("p (l c) h -> p l c h", l=2),
                    in0=acc[:, 0:320].rearrange("p (l c h) -> p l c h",
                                                l=2, c=H2),
                    in1=rec[:].rearrange("p (l h) -> p l h", l=2).unsqueeze(2)
                    .to_broadcast([P, 2, H2, HEADS]),
                    op=OP.mult,
                )
                o64 = spool.tile([P, 2 * H2], f32, tag="o64b2")
                nc.vector.tensor_reduce(out=o64[:], in_=tmp[:],
                                        axis=mybir.AxisListType.X, op=OP.add)
                o64b = spool.tile([P, 2 * H2], f32, tag="o64c2")
                nc.vector.tensor_tensor(out=o64b[:], in0=o64[:], in1=b23_t[:],
                                        op=OP.add)
                nc.sync.dma_start(out_d[k * P : (k + 1) * P, :], o64b[:])

            acc_of = {}
            mm_done = {}
            pend = None
            ready = []

            def weight_and_aggregate_b(ch, gt, wt, ci):
                d_tot = sum(s[2] for s in ch)
                for (li, dsl) in ((0, slice(320, 325)), (1, slice(325, 330))):
                    nc.vector.tensor_tensor(
                        out=gt[:, 0:d_tot, 160 * li : 160 * li + 160].rearrange(
                            "p d (c h) -> p d c h", c=H2),
                        in0=gt[:, 0:d_tot, 160 * li : 160 * li + 160].rearrange(
                            "p d (c h) -> p d c h", c=H2),
                        in1=gt[:, 0:d_tot, dsl].unsqueeze(2).to_broadcast(
                            [P, d_tot, H2, HEADS]),
                        op=OP.mult,
                    )
                off = 0
                soff = 0
                for (k, j0, dn) in ch:
                    if k not in acc_of:
                        acc_of[k] = papool.tile([P, 336], f32, space="PSUM",
                                                name="acc", tag="acc")
                        mm_done[k] = 0
                    acc = acc_of[k]
                    segs = []
                    if direct_b[ci]:
                        segs = [gt[:, off + i, 0:TWB] for i in range(dn)]
                    else:
                        npair = dn // 2
                        if npair:
                            nc.vector.tensor_tensor(
                                out=wt[:, soff : soff + npair, 0:TWB],
                                in0=gt[:, off : off + 2 * npair - 1 : 2, 0:TWB],
                                in1=gt[:, off + 1 : off + 2 * npair : 2, 0:TWB],
                                op=OP.add)
                            segs = [wt[:, soff + i, 0:TWB] for i in range(npair)]
                            soff += npair
                        if dn % 2:
                            segs.append(gt[:, off + dn - 1, 0:TWB])
                    for seg in segs:
                        nc.tensor.matmul(acc[:, 0:TWB], ident_t[:], seg,
                                         start=(mm_done[k] == 0),
                                         stop=(mm_done[k] == n_mm_b[k] - 1),
                                         skip_group_check=True)
                        mm_done[k] += 1
                    if mm_done[k] == n_mm_b[k]:
                        ready.append((k, acc_of.pop(k)))
                    off += dn

            for ci, ch in enumerate(chunks):
                d_tot = sum(s[2] for s in ch)
                coff = chunk_cols[ci]
                gt = gpool.tile([P, SLOT_CAP, WB], f16, tag="gt2")
                nidx = P * d_tot
                nc.gpsimd.dma_gather(
                    gt[:, 0:d_tot, :], tbl2_d[:],
                    idx2_t[:, coff : coff + 8 * d_tot], nidx, nidx, WB,
                    queue_num=ci % 4,
                )
                wt = wpool.tile([P, 4, TWB], f16, tag="wt2")
                ut = spool.tile([P, SLOT_CAP, 10], f16, tag="ut23")
                off = 0
                for (k, j0, dn) in ch:
                    nc.vector.tensor_tensor(
                        out=ut[:, off : off + dn, :],
                        in0=gt[:, off : off + dn, 320:330],
                        in1=ad23[:, k, :].unsqueeze(1).to_broadcast([P, dn, 10]),
                        op=OP.add,
                    )
                    off += dn
                lt = spool.tile([P, SLOT_CAP, 10], f16, tag="lt23")
                nc.scalar.activation(lt[:, 0:d_tot, :], ut[:, 0:d_tot, :],
                                     AF.Prelu, alpha=NEG)
                nc.scalar.activation(gt[:, 0:d_tot, 320:330],
                                     lt[:, 0:d_tot, :], AF.Exp, bias=ebias[:])
                if pend is not None:
                    weight_and_aggregate_b(*pend)
                    for (k, acc) in ready:
                        epilogue_b(k, acc)
                    ready.clear()
                pend = (ch, gt, wt, ci)
            weight_and_aggregate_b(*pend)
            pend = None
            for (k, acc) in ready:
                epilogue_b(k, acc)
            ready.clear()
            wpool_cm.__exit__(None, None, None)
            gpool_cm.__exit__(None, None, None)
            spool_cm.__exit__(None, None, None)

    nc.compile()
    return nc


# ----------------------------------------------------------------------------
# entry point
# ----------------------------------------------------------------------------
def kernel(x, edge_index, W1, att_src1, att_dst1, b1,
           W2, att_src2, att_dst2, b2,
           W3, att_src3, att_dst3, b3):
    global _compiled
    from concourse.bass_utils import run_bass_kernel_spmd

    x = np.asarray(x, np.float32)
    edge_index = np.asarray(edge_index)

    idx1_all, idx2_all, meta = _prep_graph(edge_index.astype(np.int64))
    chunks, idx_cols = meta["chunks"], meta["idx_cols"]
    D_band = meta["D_band"]

    key = (tuple(tuple(ch) for ch in chunks), idx_cols, D_band)
    if _compiled is None or _compiled[0] != key:
        nc = _build_program(chunks, idx_cols, D_band)
        _compiled = (key, nc)
    nc = _compiled[1]

    # host-side weight augmentation (payload columns in c-major order)
    w1s, w1dst = _w_aug(np.asarray(W1, np.float32), np.asarray(att_src1),
                        np.asarray(att_dst1), HEADS, H1)
    w1big = np.zeros((FIN, WB), np.float32)
    w1big[:, 0:320] = _cmajor(np.asarray(W1, np.float32), HEADS, H1)
    w1big[:, 320:325] = w1s
    w1big[:, 325:330] = w1dst

    w2s, w2dst = _w_aug(np.asarray(W2, np.float32), np.asarray(att_src2),
                        np.asarray(att_dst2), HEADS, H2)
    w3s, w3dst = _w_aug(np.asarray(W3, np.float32), np.asarray(att_src3),
                        np.asarray(att_dst3), HEADS, H2)
    w2big = np.zeros((H1, WB), np.float32)
    w2big[:, 0:160] = _cmajor(np.asarray(W2, np.float32), HEADS, H2)
    w2big[:, 160:320] = _cmajor(np.asarray(W3, np.float32), HEADS, H2)
    w2big[:, 320:325] = w2s
    w2big[:, 325:330] = w3s
    w2big[:, 330:335] = w2dst
    w2big[:, 335:340] = w3dst

    # fp8 sentinel row (table 1): payload 0, fp16 logit halves = -1e4
    sent_row = np.zeros((1, WBB), np.int8)
    sent_row.view(np.float16)[0, 160:170] = -1e4
    # fp16 sentinel row (table 2)
    sent2_row = np.zeros((1, WB), np.float16)
    sent2_row[0, 320:340] = -1e4

    core, band, slot = meta["core"], meta["band"], meta["slot"]
    in_maps = []
    for c in range(NCORE):
        m = core == c
        xT = np.zeros((MLOC, FIN), np.float32)
        xT[band[m] * P + slot[m]] = x[m]
        in_maps.append({
            "xT": np.ascontiguousarray(xT.T).astype(ml_dtypes.bfloat16),
            "w1big": w1big.astype(ml_dtypes.bfloat16),
            "w2big": w2big, "sent": sent_row,
            "sent2": sent2_row,
            "idx1": np.ascontiguousarray(idx1_all[c]),
            "idx2": np.ascontiguousarray(idx2_all[c]),
            "b1r": np.tile(np.asarray(b1, np.float32)[None, :], (P, 1)),
            "b23r": np.tile(np.concatenate([np.asarray(b2, np.float32),
                                            np.asarray(b3, np.float32)])[None, :],
                            (P, 1)),
        })

    global LAST_RESULTS
    res = run_bass_kernel_spmd(nc, in_maps, core_ids=list(range(NCORE)),
                               trace=TRACE, tmpdir=TRACE_DIR)
    LAST_RESULTS = res

    mu = np.empty((N, H2), np.float32)
    lv = np.empty((N, H2), np.float32)
    rows = band * P + slot
    for c in range(NCORE):
        m = core == c
        o = res.results[c]["out"][rows[m]]
        mu[m] = o[:, 0:H2]
        lv[m] = o[:, H2 : 2 * H2]
    return mu, mu.copy(), lv
